# revision 14
# baseline (speedup 1.0000x reference)
"""CGCNN / GENConv GNN message-passing kernel for 8 Trainium2 NeuronCores.

Strategy (dst-sharded edge parallel):
  - Host sorts edges by dst and shards them by dst node range: core k owns
    nodes [k*3750, (k+1)*3750) and every edge pointing into that range.
    Segment softmax/sums therefore never cross cores.
  - Each layer: every core gathers h[src] for its edges from a replicated
    node table in its local DRAM (dma_gather, 512B rows), computes
    msg = relu(h_src + ea) + eps, e = exp(t*msg), me = msg*e, and
    segment-reduces [sum e | sum me] per 128-node window with a one-hot
    matmul accumulated in PSUM.  alpha-normalization folds into a single
    per-node divide: agg = (sum me) / (sum e + eps)  (exactly equal to the
    reference softmax aggregation up to ~1e-16: the max-subtraction in the
    reference cancels algebraically and logits here are O(1)).
  - Node MLP / LayerNorm runs data-parallel on the core's own node shard.
  - The updated table (conv input of the next layer) is AllGather'ed
    across the 8 cores (~1.9MB per rank).

kernel(**inputs) takes the FULL inputs and returns the FULL [30000, 10]
output; sharding + compilation happen inside (compiled program is cached).
"""

import os
import sys

sys.path.insert(0, "/opt/trn_rl_repo")

ME_ENGINE = os.environ.get("K_ME_ENGINE", "gpsimd")  # gpsimd | vector
NO_GATHER = os.environ.get("K_NO_GATHER", "0") == "1"
NO_ONEHOT = os.environ.get("K_NO_ONEHOT", "0") == "1"
SKIP_MLP = os.environ.get("K_SKIP_MLP", "0") == "1"
NO_INPLACE = os.environ.get("K_NO_INPLACE", "0") == "1"
MLP_STAGE = int(os.environ.get("K_MLP_STAGE", "9"))
LN_STAGE = int(os.environ.get("K_LN_STAGE", "9"))

import numpy as np

import concourse.bacc as bacc
import concourse.bass as bass
import concourse.mybir as mybir
import concourse.tile as tile
from concourse.bass_utils import run_bass_kernel_spmd
from concourse.library_config import mlp as mlp_lib

F32 = mybir.dt.float32
AF = mybir.ActivationFunctionType
ALU = mybir.AluOpType

MSG_EPS = 1e-7
SM_EPS = 1e-16
LN_EPS = 1e-5

# problem dims (hardcoded per harness contract)
N_NODES = 30000
N_EDGES = 480000
F_IN = 64
F_EDGE = 16
HID = 128
N_LAYERS = 3
N_CLASSES = 10
CORES = 8


# --------------------------------------------------------------------------
# host-side sharding / packing
# --------------------------------------------------------------------------

def _prep_edges(edge_index, edge_attr, n_nodes, cores, npc, win):
    """Sort edges by dst, shard by dst range, pack per (core, window, tile).

    win = nodes per window (128).  Returns (T, per-core dict arrays).
    """
    src = edge_index[0].astype(np.int64)
    dst = edge_index[1].astype(np.int64)
    order = np.argsort(dst, kind="stable")
    src = src[order]
    dst = dst[order]
    attr = edge_attr[order]

    W = (npc + win - 1) // win  # windows per core
    # window id of each edge globally: core * W + (local node // win)
    core_of = dst // npc
    wloc = (dst - core_of * npc) // win
    gwin = core_of * W + wloc
    # edges are sorted by dst so gwin is non-decreasing
    counts = np.bincount(gwin, minlength=cores * W)
    T = int(np.ceil(counts.max() / 128.0))
    T = max(T, 1)
    epw = T * 128  # padded edges per window
    EPAD = W * epw

    starts = np.zeros(cores * W + 1, np.int64)
    np.cumsum(counts, out=starts[1:])

    fe = attr.shape[1]
    src_pad = np.zeros((cores, W, epw), np.int64)
    dstloc_pad = np.full((cores, W, epw), -1.0, np.float32)
    attr_pad = np.zeros((cores, W, epw, fe), np.float32)
    for c in range(cores):
        for w in range(W):
            g = c * W + w
            s, e = starts[g], starts[g + 1]
            n = e - s
            src_pad[c, w, :n] = src[s:e]
            dstloc_pad[c, w, :n] = (dst[s:e] - (c * npc + w * win)).astype(
                np.float32
            )
            attr_pad[c, w, :n, :] = attr[s:e]

    # gather index layout: [128, W*T*8] int16, idx i of window w at
    # partition i%16 (replicated x8), column w*T*8 + i//16
    sp = src_pad.reshape(cores, W, T * 8, 16)
    gidx16 = np.transpose(sp, (0, 3, 1, 2)).reshape(cores, 16, W * T * 8)
    gidx = np.tile(gidx16, (1, 8, 1)).astype(np.int16)

    # dstloc: [128, W*T] f32, tile j=w*T+g column, partition = edge offset
    dl = dstloc_pad.reshape(cores, W, T, 128)
    dstloc = np.ascontiguousarray(
        np.transpose(dl, (0, 3, 1, 2)).reshape(cores, 128, W * T)
    )

    # attrT: [17, EPAD]: rows 0..15 features (transposed), row 16 = ones
    ap = attr_pad.reshape(cores, EPAD, fe)
    attrT = np.empty((cores, fe + 1, EPAD), np.float32)
    attrT[:, :fe, :] = np.transpose(ap, (0, 2, 1))
    attrT[:, fe, :] = 1.0
    return T, W, gidx, dstloc, np.ascontiguousarray(attrT)


def _prep_inputs(inputs, cores=CORES):
    """Build the 8 per-core input maps (and shared weight arrays)."""
    x = np.asarray(inputs["x"], np.float32)
    edge_attr = np.asarray(inputs["edge_attr"], np.float32)
    edge_index = np.asarray(inputs["edge_index"])
    n_nodes, fin = x.shape
    npc = n_nodes // cores
    win = 128

    T, W, gidx, dstloc, attrT = _prep_edges(
        edge_index, edge_attr, n_nodes, cores, npc, win
    )

    L = int(np.asarray(inputs["t"]).shape[0])
    hid = np.asarray(inputs["node_enc_w"]).shape[1]

    # xT per core: [fin+1, npc] with ones row
    xs = x.reshape(cores, npc, fin)
    xT = np.empty((cores, fin + 1, npc), np.float32)
    xT[:, :fin, :] = np.transpose(xs, (0, 2, 1))
    xT[:, fin, :] = 1.0

    wnode = np.concatenate(
        [np.asarray(inputs["node_enc_w"], np.float32),
         np.asarray(inputs["node_enc_b"], np.float32)[None, :]], 0
    )
    wenc = np.concatenate(
        [np.asarray(inputs["edge_enc_w"], np.float32),
         np.asarray(inputs["edge_enc_b"], np.float32)[None, :]], 0
    )
    w1 = np.ascontiguousarray(inputs["mlp1_w"], np.float32)      # [L,H,2H]
    w2 = np.ascontiguousarray(inputs["mlp2_w"], np.float32)      # [L,2H,H]
    b1 = np.ascontiguousarray(
        np.asarray(inputs["mlp1_b"], np.float32).reshape(1, -1))  # [1,L*2H]
    b2 = np.ascontiguousarray(
        np.asarray(inputs["mlp2_b"], np.float32).reshape(1, -1))  # [1,L*H]
    g1bc = np.ascontiguousarray(
        np.broadcast_to(np.asarray(inputs["mlp_ln_g"], np.float32)[:, None, :],
                        (L, 128, 2 * hid)))
    bb1bc = np.ascontiguousarray(
        np.broadcast_to(np.asarray(inputs["mlp_ln_b"], np.float32)[:, None, :],
                        (L, 128, 2 * hid)))
    ngbc = np.ascontiguousarray(
        np.broadcast_to(np.asarray(inputs["norm_g"], np.float32)[:, None, :],
                        (L, 128, hid)))
    nbbc = np.ascontiguousarray(
        np.broadcast_to(np.asarray(inputs["norm_b"], np.float32)[:, None, :],
                        (L, 128, hid)))
    tcol = np.ascontiguousarray(
        np.broadcast_to(np.asarray(inputs["t"], np.float32)[None, :], (128, L)))
    linw = np.ascontiguousarray(inputs["lin_w"], np.float32)
    linb = np.ascontiguousarray(
        np.asarray(inputs["lin_b"], np.float32)[None, :])
    iota = np.ascontiguousarray(
        np.broadcast_to(np.arange(128, dtype=np.float32)[None, :], (128, 128)))
    ident = np.eye(128, dtype=np.float32)

    shared = dict(wnode=wnode, wenc=wenc, w1=w1, w2=w2, b1=b1, b2=b2,
                  g1bc=g1bc, bb1bc=bb1bc, ngbc=ngbc, nbbc=nbbc, tcol=tcol,
                  linw=linw, linb=linb, iota=iota, ident=ident)
    in_maps = []
    for c in range(cores):
        m = dict(shared)
        m["xT"] = np.ascontiguousarray(xT[c])
        m["attrT"] = attrT[c]
        m["gidx"] = np.ascontiguousarray(gidx[c])
        m["dstloc"] = dstloc[c]
        in_maps.append(m)
    return T, W, npc, in_maps


# --------------------------------------------------------------------------
# device program
# --------------------------------------------------------------------------

def build_program(T, W, npc, n_nodes=N_NODES, cores=CORES, fin=F_IN,
                  fe=F_EDGE, hid=HID, L=N_LAYERS, ncls=N_CLASSES):
    EPAD = W * T * 128
    H2 = 2 * hid
    nc = bacc.Bacc("TRN2", target_bir_lowering=False, debug=False,
                   num_devices=cores)

    d = nc.dram_tensor
    xT_d = d("xT", [fin + 1, npc], F32, kind="ExternalInput")
    attrT_d = d("attrT", [fe + 1, EPAD], F32, kind="ExternalInput")
    gidx_d = d("gidx", [128, W * T * 8], mybir.dt.int16, kind="ExternalInput")
    dstloc_d = d("dstloc", [128, W * T], F32, kind="ExternalInput")
    wnode_d = d("wnode", [fin + 1, hid], F32, kind="ExternalInput")
    wenc_d = d("wenc", [fe + 1, hid], F32, kind="ExternalInput")
    w1_d = d("w1", [L, hid, H2], F32, kind="ExternalInput")
    w2_d = d("w2", [L, H2, hid], F32, kind="ExternalInput")
    b1_d = d("b1", [1, L * H2], F32, kind="ExternalInput")
    b2_d = d("b2", [1, L * hid], F32, kind="ExternalInput")
    g1bc_d = d("g1bc", [L, 128, H2], F32, kind="ExternalInput")
    bb1bc_d = d("bb1bc", [L, 128, H2], F32, kind="ExternalInput")
    ngbc_d = d("ngbc", [L, 128, hid], F32, kind="ExternalInput")
    nbbc_d = d("nbbc", [L, 128, hid], F32, kind="ExternalInput")
    tcol_d = d("tcol", [128, L], F32, kind="ExternalInput")
    linw_d = d("linw", [hid, ncls], F32, kind="ExternalInput")
    linb_d = d("linb", [1, ncls], F32, kind="ExternalInput")
    iota_d = d("iota", [128, 128], F32, kind="ExternalInput")
    ident_d = d("ident", [128, 128], F32, kind="ExternalInput")
    out_d = d("out", [npc, ncls], F32, kind="ExternalOutput")

    rg = [list(range(cores))]

    with tile.TileContext(nc) as tc:
        nc.gpsimd.load_library(mlp_lib)
        with (
            tc.tile_pool(name="const", bufs=1) as cp,
            tc.tile_pool(name="sbw", bufs=2) as sbw,       # window tiles
            tc.tile_pool(name="sbm", bufs=2) as sbm,       # MLP scratch
            tc.tile_pool(name="psq", bufs=2, space="PSUM") as psq,
            tc.tile_pool(name="psa", bufs=2, space="PSUM") as psa,
            tc.tile_pool(name="psu", bufs=1, space="PSUM") as psu,
            tc.tile_pool(name="pst", bufs=2, space="PSUM") as pst,
            tc.tile_pool(name="pso", bufs=1, space="PSUM") as pso,
            tc.tile_pool(name="dram", bufs=2, space="DRAM") as dp,
            tc.tile_pool(name="dram1", bufs=1, space="DRAM") as dp1,
        ):
            # ---------------- constants / weights to SBUF ----------------
            def load(name, dram_ap, shape, rearr=None, **kw):
                t = cp.tile(shape, F32, tag=name)
                src = dram_ap if rearr is None else dram_ap.rearrange(rearr, **kw)
                nc.sync.dma_start(t[:], src)
                return t

            wnode_s = load("wnode", wnode_d[:, :], [fin + 1, hid])
            wenc_s = load("wenc", wenc_d[:, :], [fe + 1, hid])
            w1_s = load("w1", w1_d[:, :, :], [hid, L, H2], "l k n -> k l n")
            w2_s = load("w2", w2_d[:, :, :], [128, L, 2, hid],
                        "l (h k) n -> k l h n", h=2)
            b1_s = load("b1", b1_d[:, :], [1, L * H2])
            b2_s = load("b2", b2_d[:, :], [1, L * hid])
            g1_s = load("g1", g1bc_d[:, :, :], [128, L, H2], "l p n -> p l n")
            bb1_s = load("bb1", bb1bc_d[:, :, :], [128, L, H2], "l p n -> p l n")
            ng_s = load("ng", ngbc_d[:, :, :], [128, L, hid], "l p n -> p l n")
            nb_s = load("nb", nbbc_d[:, :, :], [128, L, hid], "l p n -> p l n")
            tcol_s = load("tcol", tcol_d[:, :], [128, L])
            linw_s = load("linw", linw_d[:, :], [hid, ncls])
            linb_s = load("linb", linb_d[:, :], [1, ncls])
            iota_s = load("iota", iota_d[:, :], [128, 128])
            ident_s = load("ident", ident_d[:, :], [128, 128])
            dstloc_s = load("dstloc", dstloc_d[:, :], [128, W * T])
            gidx_s = cp.tile([128, W * T * 8], mybir.dt.int16, tag="gidx")
            nc.sync.dma_start(gidx_s[:], gidx_d[:, :])

            eps_col = cp.tile([128, 1], F32, tag="epsc")
            nc.vector.memset(eps_col[:], MSG_EPS)
            sm_col = cp.tile([128, 1], F32, tag="smc")
            nc.vector.memset(sm_col[:], SM_EPS)
            ln_col = cp.tile([128, 1], F32, tag="lnc")
            nc.vector.memset(ln_col[:], LN_EPS)
            ones_row = cp.tile([1, 128], F32, tag="ones")
            nc.vector.memset(ones_row[:], 1.0)

            h_state = dp1.tile([npc, hid], F32)

            # ---------------- helpers ----------------
            def layer_norm_relu(dst, src_ap, nfeat, g_ap, b_ap, sq_tag):
                """dst <- relu(LN(src) * g + b).  src may be PSUM."""
                ssum = sbm.tile([128, 1], F32, tag="lnsum")
                nc.vector.reduce_sum(ssum[:], src_ap, axis=mybir.AxisListType.X)
                mcol = sbm.tile([128, 1], F32, tag="lnm")
                nc.scalar.mul(mcol[:], ssum[:], 1.0 / nfeat)
                xm = sbm.tile([128, nfeat], F32, tag="lnxm" + sq_tag)
                if LN_STAGE == 1:
                    nc.vector.tensor_copy(xm[:], src_ap)
                    nc.scalar.activation(dst, xm[:], AF.Relu)
                    return
                nc.vector.tensor_scalar_sub(xm[:], src_ap, mcol[:])
                if LN_STAGE == 2:
                    nc.scalar.activation(dst, xm[:], AF.Relu)
                    return
                sq = sbm.tile([128, nfeat], F32, tag="lnsq" + sq_tag)
                vsum = sbm.tile([128, 1], F32, tag="lnv")
                nc.vector.tensor_mul(sq[:], xm[:], xm[:])
                nc.vector.reduce_sum(vsum[:], sq[:], axis=mybir.AxisListType.X)
                if LN_STAGE == 3:
                    nc.scalar.activation(dst, sq[:], AF.Relu)
                    return
                lnv = sbm.tile([128, 1], F32, tag="lnlnv")
                nc.scalar.activation(lnv[:], vsum[:], AF.Ln, bias=ln_col[:],
                                     scale=1.0 / nfeat)
                rstd = sbm.tile([128, 1], F32, tag="lnrstd")
                nc.scalar.activation(rstd[:], lnv[:], AF.Exp, scale=-0.5)
                if LN_STAGE == 4:
                    nc.vector.tensor_scalar_mul(xm[:], xm[:], rstd[:])
                    nc.scalar.activation(dst, xm[:], AF.Relu)
                    return
                y = sbm.tile([128, nfeat], F32, tag="lny" + sq_tag)
                nc.vector.tensor_scalar_mul(y[:], xm[:], rstd[:])
                if NO_INPLACE:
                    y2 = sbm.tile([128, nfeat], F32, tag="lnyy" + sq_tag)
                    nc.vector.tensor_mul(y2[:], y[:], g_ap)
                    y3 = sbm.tile([128, nfeat], F32, tag="lnyz" + sq_tag)
                    nc.vector.tensor_add(y3[:], y2[:], b_ap)
                    nc.scalar.activation(dst, y3[:], AF.Relu)
                else:
                    nc.vector.tensor_mul(y[:], y[:], g_ap)
                    nc.vector.tensor_add(y[:], y[:], b_ap)
                    nc.scalar.activation(dst, y[:], AF.Relu)

            def transpose128(src_ap, tag):
                """PE transpose [128,128] -> SBUF."""
                pt = pst.tile([128, 128], F32, tag="pt")
                nc.tensor.transpose(pt[:], src_ap, ident_s[:])
                st = sbm.tile([128, 128], F32, tag=tag)
                nc.vector.tensor_copy(st[:], pt[:])
                return st

            # ---------------- encode phase: h0 = x @ wnode ----------------
            ag = dp.tile([npc, hid], F32, tag="ag")
            for w in range(W):
                base = w * 128
                ws = min(128, npc - base)
                xts = sbm.tile([fin + 1, 128], F32, tag="xts")
                nc.sync.dma_start(xts[:, :ws], xT_d[:, base:base + ws])
                ph = pso.tile([128, hid], F32, tag="po")
                nc.tensor.matmul(ph[:ws, :], xts[:, :ws], wnode_s[:],
                                 start=True, stop=True)
                h0 = sbm.tile([128, hid], F32, tag="h0")
                nc.vector.tensor_copy(h0[:ws, :], ph[:ws, :])
                nc.sync.dma_start(ag[base:base + ws, :], h0[:ws, :])

            table = dp.tile([n_nodes, hid], F32, tag="table")
            nc.gpsimd.collective_compute(
                "AllGather", ALU.bypass, ins=[ag.opt()], outs=[table.opt()],
                replica_groups=rg)

            # ---------------- conv layers ----------------
            NQ = (T + 3) // 4  # quads of <=4 tiles

            for li in range(L):
                ag_next = (dp.tile([npc, hid], F32, tag="ag", name=f"agn{li}")
                           if li < L - 1 else None)
                for w in range(W):
                    base = w * 128
                    ws = min(128, npc - base)
                    jw = w * T

                    # gather h[src] for this window: [128, T, 128]
                    # (split into <=1024-index chunks: the SWDGE gather
                    # ucode wedges the device above ~1024 descriptors)
                    hsrc = sbw.tile([128, T, 128], F32, tag="hsrc")
                    if NO_GATHER:
                        nc.vector.memset(hsrc[:], 0.01)
                    else:
                        for c0 in range(0, T, 8):
                            ct = min(8, T - c0)
                            nc.gpsimd.dma_gather(
                                hsrc[:, c0:c0 + ct, :], table[:, :],
                                gidx_s[:, (w * T + c0) * 8:
                                       (w * T + c0 + ct) * 8],
                                ct * 128, ct * 128, hid)
                    attrs = sbw.tile([fe + 1, T, 128], F32, tag="attrs")
                    nc.sync.dma_start(
                        attrs[:], attrT_d[:, w * T * 128:(w + 1) * T * 128])

                    msg = sbw.tile([128, T, 128], F32, tag="msg")
                    em = sbw.tile([128, 2, T, 128], F32, tag="em", bufs=1)
                    # quads: ea matmul + gathered-h add (identity matmul)
                    for q in range(NQ):
                        q0 = q * 4
                        qs = min(4, T - q0)
                        pq = psq.tile([128, 4, 128], F32, tag="pq")
                        for j in range(qs):
                            nc.tensor.matmul(
                                pq[:, j, :], attrs[:, q0 + j, :], wenc_s[:],
                                start=(j == 0), stop=False,
                                skip_group_check=True)
                        nc.tensor.matmul(
                            pq[:, :qs, :], ident_s[:], hsrc[:, q0:q0 + qs, :],
                            start=False, stop=True, skip_group_check=True)
                        # msg = relu(ea + h_src + eps)
                        nc.scalar.activation(
                            msg[:, q0:q0 + qs, :], pq[:, :qs, :], AF.Relu,
                            bias=eps_col[:])
                    # e = exp(t * msg) ; me = msg * e
                    nc.scalar.activation(
                        em[:, 0, :, :], msg[:], AF.Exp,
                        scale=tcol_s[:, li:li + 1])
                    me_eng = nc.gpsimd if ME_ENGINE == "gpsimd" else nc.vector
                    me_eng.tensor_tensor(
                        em[:, 1, :, :], msg[:], em[:, 0, :, :], op=ALU.mult)
                    # one-hot S for the whole window
                    S = sbw.tile([128, T, 128], F32, tag="S")
                    if NO_ONEHOT:
                        nc.vector.memset(S[:], 0.0)
                    else:
                        iota_b = iota_s[:].rearrange(
                            "p (o f) -> p o f", o=1).broadcast_to([128, T, 128])
                        dl_b = dstloc_s[:, jw:jw + T].rearrange(
                            "p (t o) -> p t o", o=1).broadcast_to([128, T, 128])
                        nc.vector.tensor_tensor(S[:], iota_b, dl_b,
                                                op=ALU.is_equal)
                    # segment accumulate [sum e | sum me] -> [128, 256] psum
                    acc = psa.tile([128, 2, hid], F32, tag="acc")
                    for g in range(T):
                        nc.tensor.matmul(
                            acc[:, :, :], S[:, g, :], em[:, :, g, :],
                            start=(g == 0), stop=(g == T - 1))

                    # agg = (sum me) / (sum e + eps)  via exp(-ln(se+eps))
                    lnse = sbm.tile([128, hid], F32, tag="lnse")
                    nc.scalar.activation(lnse[:], acc[:, 0, :], AF.Ln,
                                         bias=sm_col[:])
                    rcse = sbm.tile([128, hid], F32, tag="rcse")
                    nc.scalar.activation(rcse[:], lnse[:], AF.Exp, scale=-1.0)
                    z = sbm.tile([128, hid], F32, tag="z")
                    nc.vector.tensor_mul(z[:], acc[:, 1, :], rcse[:])
                    # z += conv input rows (this core's shard of table source)
                    zin = sbm.tile([128, hid], F32, tag="zin")
                    nc.sync.dma_start(zin[:ws, :], ag[base:base + ws, :])
                    if NO_INPLACE:
                        z2t = sbm.tile([128, hid], F32, tag="z2t")
                        nc.vector.tensor_add(z2t[:], z[:], zin[:])
                        z = z2t
                    else:
                        nc.vector.tensor_add(z[:], z[:], zin[:])

                    if SKIP_MLP:
                        nc.sync.dma_start(out_d[base:base + ws, :],
                                          z[:ws, :ncls])
                        continue

                    # ---- MLP: relu(LN(z@w1+b1))@w2+b2 ----
                    zT = transpose128(z[:], "zT")
                    if MLP_STAGE == 1:
                        nc.sync.dma_start(out_d[base:base + ws, :],
                                          zT[:ws, :ncls])
                        continue
                    pu = psu.tile([128, H2], F32, tag="pu")
                    nc.tensor.matmul(pu[:], zT[:], w1_s[:, li, :],
                                     start=True, stop=False,
                                     skip_group_check=True)
                    nc.tensor.matmul(pu[:], ones_row[:],
                                     b1_s[:, li * H2:(li + 1) * H2],
                                     start=False, stop=True,
                                     skip_group_check=True)
                    if MLP_STAGE == 2:
                        uu = sbm.tile([128, ncls], F32, tag="uu")
                        nc.vector.tensor_copy(uu[:], pu[:, :ncls])
                        nc.sync.dma_start(out_d[base:base + ws, :],
                                          uu[:ws, :])
                        continue
                    r = sbm.tile([128, H2], F32, tag="r")
                    layer_norm_relu(r[:], pu[:], H2, g1_s[:, li, :],
                                    bb1_s[:, li, :], "a")
                    if MLP_STAGE == 3:
                        nc.sync.dma_start(out_d[base:base + ws, :],
                                          r[:ws, :ncls])
                        continue
                    rT0 = transpose128(r[:, 0:128], "rT0")
                    rT1 = transpose128(r[:, 128:256], "rT1")
                    po = pso.tile([128, hid], F32, tag="po")
                    nc.tensor.matmul(po[:], rT0[:], w2_s[:, li, 0, :],
                                     start=True, stop=False,
                                     skip_group_check=True)
                    nc.tensor.matmul(po[:], rT1[:], w2_s[:, li, 1, :],
                                     start=False, stop=False,
                                     skip_group_check=True)
                    nc.tensor.matmul(po[:], ones_row[:],
                                     b2_s[:, li * hid:(li + 1) * hid],
                                     start=False, stop=True,
                                     skip_group_check=True)

                    if MLP_STAGE == 4:
                        oo = sbm.tile([128, ncls], F32, tag="oo")
                        nc.vector.tensor_copy(oo[:], po[:, :ncls])
                        nc.sync.dma_start(out_d[base:base + ws, :],
                                          oo[:ws, :])
                        continue
                    # ---- layer epilogue ----
                    hcur = sbm.tile([128, hid], F32, tag="hcur")
                    if li == 0:
                        nc.vector.tensor_copy(hcur[:], po[:])
                    else:
                        hprev = sbm.tile([128, hid], F32, tag="hprev")
                        nc.sync.dma_start(hprev[:ws, :],
                                          h_state[base:base + ws, :])
                        nc.vector.tensor_add(hcur[:], po[:], hprev[:])
                    if li < L - 1:
                        nc.sync.dma_start(h_state[base:base + ws, :],
                                          hcur[:ws, :])
                        # z for next layer: relu(LN(h; norm[li+1]))
                        znext = sbm.tile([128, hid], F32, tag="znext")
                        layer_norm_relu(znext[:], hcur[:], hid,
                                        ng_s[:, li + 1, :], nb_s[:, li + 1, :],
                                        "b")
                        nc.sync.dma_start(ag_next[base:base + ws, :],
                                          znext[:ws, :])
                    else:
                        # final: relu(LN(h; norm[0])) @ lin_w + lin_b
                        fin_t = sbm.tile([128, hid], F32, tag="fin")
                        layer_norm_relu(fin_t[:], hcur[:], hid,
                                        ng_s[:, 0, :], nb_s[:, 0, :], "b")
                        finT = transpose128(fin_t[:], "finT")
                        pc = pso.tile([128, ncls], F32, tag="po")
                        nc.tensor.matmul(pc[:], finT[:], linw_s[:],
                                         start=True, stop=False,
                                         skip_group_check=True)
                        nc.tensor.matmul(pc[:], ones_row[:], linb_s[:],
                                         start=False, stop=True,
                                         skip_group_check=True)
                        ow = sbm.tile([128, ncls], F32, tag="ow")
                        nc.vector.tensor_copy(ow[:], pc[:])
                        nc.sync.dma_start(out_d[base:base + ws, :],
                                          ow[:ws, :])

                if li < L - 1:
                    table = dp.tile([n_nodes, hid], F32, tag="table")
                    nc.gpsimd.collective_compute(
                        "AllGather", ALU.bypass, ins=[ag_next.opt()],
                        outs=[table.opt()], replica_groups=rg)
                    ag = ag_next

    nc.compile()
    return nc


# --------------------------------------------------------------------------
# entry point
# --------------------------------------------------------------------------

_CACHE = {}


def _get_program(T, W, npc):
    key = (T, W, npc)
    if key not in _CACHE:
        _CACHE[key] = build_program(T, W, npc)
    return _CACHE[key]


def _install_ntff_hook():
    """Bridge trn_agent_boot's ctypes NTFF profiler into antenv.axon_hooks
    (absent from this image) so run_bass_kernel_spmd(trace=True) works."""
    import types

    if "antenv.axon_hooks" in sys.modules:
        return
    try:
        sys.path.insert(0, "/root/.axon_site")
        from trn_agent_boot.trn_boot import _ntff_profile_via_ctypes

        hook = _ntff_profile_via_ctypes("/opt/axon/libaxon_pjrt.so")
    except Exception:
        hook = None
    m = types.ModuleType("antenv.axon_hooks")
    state = {"hook": hook}
    m.get_axon_ntff_profile_hook = lambda: state["hook"]
    m.set_axon_ntff_profile_hook = lambda h: state.update(hook=h)
    sys.modules["antenv.axon_hooks"] = m
    import antenv

    antenv.axon_hooks = m


def run(inputs, trace=False):
    if trace:
        _install_ntff_hook()
    T, W, npc, in_maps = _prep_inputs(inputs)
    nc = _get_program(T, W, npc)
    res = run_bass_kernel_spmd(nc, in_maps, list(range(CORES)), trace=trace)
    out = np.concatenate([res.results[c]["out"] for c in range(CORES)], axis=0)
    return out, res


def kernel(**inputs) -> np.ndarray:
    out, _ = run(inputs, trace=False)
    return out


# revision 18
# speedup vs baseline: 1.6766x; 1.6766x over previous
"""CGCNN / GENConv GNN message-passing kernel for 8 Trainium2 NeuronCores.

Strategy (dst-sharded edge parallel):
  - Host sorts edges by dst and shards them by dst node range: core k owns
    nodes [k*3750, (k+1)*3750) and every edge pointing into that range.
    Segment softmax/sums therefore never cross cores.
  - Each layer: every core gathers h[src] for its edges from a replicated
    node table in its local DRAM (dma_gather, 512B rows), computes
    msg = relu(h_src + ea) + eps, e = exp(t*msg), me = msg*e, and
    segment-reduces [sum e | sum me] per 128-node window with a one-hot
    matmul accumulated in PSUM.  alpha-normalization folds into a single
    per-node divide: agg = (sum me) / (sum e + eps)  (exactly equal to the
    reference softmax aggregation up to ~1e-16: the max-subtraction in the
    reference cancels algebraically and logits here are O(1)).
  - Node MLP / LayerNorm runs data-parallel on the core's own node shard.
  - The updated table (conv input of the next layer) is AllGather'ed
    across the 8 cores (~1.9MB per rank).

kernel(**inputs) takes the FULL inputs and returns the FULL [30000, 10]
output; sharding + compilation happen inside (compiled program is cached).
"""

import os
import sys

sys.path.insert(0, "/opt/trn_rl_repo")

ME_ENGINE = os.environ.get("K_ME_ENGINE", "vector")  # gpsimd | vector
NO_GATHER = os.environ.get("K_NO_GATHER", "0") == "1"
NO_ONEHOT = os.environ.get("K_NO_ONEHOT", "0") == "1"
SKIP_MLP = os.environ.get("K_SKIP_MLP", "0") == "1"
NO_INPLACE = os.environ.get("K_NO_INPLACE", "0") == "1"
MLP_STAGE = int(os.environ.get("K_MLP_STAGE", "9"))
LN_STAGE = int(os.environ.get("K_LN_STAGE", "9"))

import numpy as np

import concourse.bacc as bacc
import concourse.bass as bass
import concourse.mybir as mybir
import concourse.tile as tile
from concourse.bass_utils import run_bass_kernel_spmd
from concourse.library_config import mlp as mlp_lib

F32 = mybir.dt.float32
F16 = mybir.dt.float16
F32R = mybir.dt.float32r
I32 = mybir.dt.int32
AF = mybir.ActivationFunctionType
ALU = mybir.AluOpType

MSG_EPS = 1e-7
SM_EPS = 1e-16
LN_EPS = 1e-5

# problem dims (hardcoded per harness contract)
N_NODES = 30000
N_EDGES = 480000
F_IN = 64
F_EDGE = 16
HID = 128
N_LAYERS = 3
N_CLASSES = 10
CORES = 8


# --------------------------------------------------------------------------
# host-side sharding / packing
# --------------------------------------------------------------------------

def _prep_edges(edge_index, edge_attr, n_nodes, cores, npc, win):
    """Sort edges by dst, shard by dst range, pack per (core, window, tile).

    win = nodes per window (128).  Returns (T, per-core dict arrays).
    """
    src = edge_index[0].astype(np.int64)
    dst = edge_index[1].astype(np.int64)
    order = np.argsort(dst, kind="stable")
    src = src[order]
    dst = dst[order]
    attr = edge_attr[order]

    W = (npc + win - 1) // win  # windows per core
    # window id of each edge globally: core * W + (local node // win)
    core_of = dst // npc
    wloc = (dst - core_of * npc) // win
    gwin = core_of * W + wloc
    # edges are sorted by dst so gwin is non-decreasing
    counts = np.bincount(gwin, minlength=cores * W)
    T = int(np.ceil(counts.max() / 128.0))
    T = max(T, 1)
    epw = T * 128  # padded edges per window
    EPAD = W * epw

    starts = np.zeros(cores * W + 1, np.int64)
    np.cumsum(counts, out=starts[1:])

    fe = attr.shape[1]
    src_pad = np.zeros((cores, W, epw), np.int64)
    dstloc_pad = np.full((cores, W, epw), -1.0, np.float32)
    attr_pad = np.zeros((cores, W, epw, fe), np.float32)
    for c in range(cores):
        for w in range(W):
            g = c * W + w
            s, e = starts[g], starts[g + 1]
            n = e - s
            src_pad[c, w, :n] = src[s:e]
            dstloc_pad[c, w, :n] = (dst[s:e] - (c * npc + w * win)).astype(
                np.float32
            )
            attr_pad[c, w, :n, :] = attr[s:e]

    # gather index layout: [128, W*T*8] int16, idx i of window w at
    # partition i%16 (replicated x8), column w*T*8 + i//16
    sp = src_pad.reshape(cores, W, T * 8, 16)
    gidx16 = np.transpose(sp, (0, 3, 1, 2)).reshape(cores, 16, W * T * 8)
    gidx = np.tile(gidx16, (1, 8, 1)).astype(np.int16)

    # dstloc: [128, W*T] f32, tile j=w*T+g column, partition = edge offset
    dl = dstloc_pad.reshape(cores, W, T, 128)
    dstloc = np.ascontiguousarray(
        np.transpose(dl, (0, 3, 1, 2)).reshape(cores, 128, W * T)
    )

    # attrT: [17, EPAD]: rows 0..15 features (transposed), row 16 = ones
    ap = attr_pad.reshape(cores, EPAD, fe)
    attrT = np.empty((cores, fe + 1, EPAD), np.float16)
    attrT[:, :fe, :] = np.transpose(ap, (0, 2, 1))
    attrT[:, fe, :] = 1.0
    return T, W, gidx, dstloc, np.ascontiguousarray(attrT)


def _prep_inputs(inputs, cores=CORES):
    """Build the 8 per-core input maps (and shared weight arrays)."""
    x = np.asarray(inputs["x"], np.float32)
    edge_attr = np.asarray(inputs["edge_attr"], np.float32)
    edge_index = np.asarray(inputs["edge_index"])
    n_nodes, fin = x.shape
    npc = n_nodes // cores
    win = 128

    T, W, gidx, dstloc, attrT = _prep_edges(
        edge_index, edge_attr, n_nodes, cores, npc, win
    )

    L = int(np.asarray(inputs["t"]).shape[0])
    hid = np.asarray(inputs["node_enc_w"]).shape[1]

    # xT per core: [fin+1, npc] with ones row
    xs = x.reshape(cores, npc, fin)
    xT = np.empty((cores, fin + 1, npc), np.float32)
    xT[:, :fin, :] = np.transpose(xs, (0, 2, 1))
    xT[:, fin, :] = 1.0

    wnode = np.concatenate(
        [np.asarray(inputs["node_enc_w"], np.float32),
         np.asarray(inputs["node_enc_b"], np.float32)[None, :]], 0
    )
    wenc = np.concatenate(
        [np.asarray(inputs["edge_enc_w"], np.float32),
         np.asarray(inputs["edge_enc_b"], np.float32)[None, :]], 0
    )
    w1 = np.ascontiguousarray(inputs["mlp1_w"], np.float32)      # [L,H,2H]
    w2 = np.ascontiguousarray(inputs["mlp2_w"], np.float32)      # [L,2H,H]
    b1 = np.ascontiguousarray(
        np.asarray(inputs["mlp1_b"], np.float32).reshape(1, -1))  # [1,L*2H]
    b2 = np.ascontiguousarray(
        np.asarray(inputs["mlp2_b"], np.float32).reshape(1, -1))  # [1,L*H]
    g1bc = np.ascontiguousarray(
        np.broadcast_to(np.asarray(inputs["mlp_ln_g"], np.float32)[:, None, :],
                        (L, 128, 2 * hid)))
    bb1bc = np.ascontiguousarray(
        np.broadcast_to(np.asarray(inputs["mlp_ln_b"], np.float32)[:, None, :],
                        (L, 128, 2 * hid)))
    ngbc = np.ascontiguousarray(
        np.broadcast_to(np.asarray(inputs["norm_g"], np.float32)[:, None, :],
                        (L, 128, hid)))
    nbbc = np.ascontiguousarray(
        np.broadcast_to(np.asarray(inputs["norm_b"], np.float32)[:, None, :],
                        (L, 128, hid)))
    tcol = np.ascontiguousarray(
        np.broadcast_to(np.asarray(inputs["t"], np.float32)[None, :], (128, L)))
    linw = np.ascontiguousarray(inputs["lin_w"], np.float32)
    linb = np.ascontiguousarray(
        np.asarray(inputs["lin_b"], np.float32)[None, :])
    iota = np.ascontiguousarray(
        np.broadcast_to(np.arange(128, dtype=np.float32)[None, :], (128, 128)))
    ident = np.eye(128, dtype=np.float32)

    wench = wenc.astype(np.float16)
    w1h = w1.astype(np.float16)
    w2h = w2.astype(np.float16)
    linwh = linw.astype(np.float16)
    shared = dict(wnode=wnode, wench=wench, w1h=w1h, w2h=w2h, b1=b1, b2=b2,
                  g1bc=g1bc, bb1bc=bb1bc, ngbc=ngbc, nbbc=nbbc, tcol=tcol,
                  linwh=linwh, linb=linb, iota=iota, ident=ident)
    in_maps = []
    for c in range(cores):
        m = dict(shared)
        m["xT"] = np.ascontiguousarray(xT[c])
        m["attrT"] = attrT[c]
        m["gidx"] = np.ascontiguousarray(gidx[c])
        m["dstloc"] = dstloc[c]
        in_maps.append(m)
    return T, W, npc, in_maps


# --------------------------------------------------------------------------
# device program
# --------------------------------------------------------------------------

def build_program(T, W, npc, n_nodes=N_NODES, cores=CORES, fin=F_IN,
                  fe=F_EDGE, hid=HID, L=N_LAYERS, ncls=N_CLASSES):
    EPAD = W * T * 128
    H2 = 2 * hid
    nc = bacc.Bacc("TRN2", target_bir_lowering=False, debug=False,
                   num_devices=cores)

    d = nc.dram_tensor
    xT_d = d("xT", [fin + 1, npc], F32, kind="ExternalInput")
    attrT_d = d("attrT", [fe + 1, EPAD], F16, kind="ExternalInput")
    gidx_d = d("gidx", [128, W * T * 8], mybir.dt.int16, kind="ExternalInput")
    dstloc_d = d("dstloc", [128, W * T], F32, kind="ExternalInput")
    wnode_d = d("wnode", [fin + 1, hid], F32, kind="ExternalInput")
    wenc_d = d("wench", [fe + 1, hid], F16, kind="ExternalInput")
    w1_d = d("w1h", [L, hid, H2], F16, kind="ExternalInput")
    w2_d = d("w2h", [L, H2, hid], F16, kind="ExternalInput")
    b1_d = d("b1", [1, L * H2], F32, kind="ExternalInput")
    b2_d = d("b2", [1, L * hid], F32, kind="ExternalInput")
    g1bc_d = d("g1bc", [L, 128, H2], F32, kind="ExternalInput")
    bb1bc_d = d("bb1bc", [L, 128, H2], F32, kind="ExternalInput")
    ngbc_d = d("ngbc", [L, 128, hid], F32, kind="ExternalInput")
    nbbc_d = d("nbbc", [L, 128, hid], F32, kind="ExternalInput")
    tcol_d = d("tcol", [128, L], F32, kind="ExternalInput")
    linw_d = d("linwh", [hid, ncls], F16, kind="ExternalInput")
    linb_d = d("linb", [1, ncls], F32, kind="ExternalInput")
    iota_d = d("iota", [128, 128], F32, kind="ExternalInput")
    ident_d = d("ident", [128, 128], F32, kind="ExternalInput")
    out_d = d("out", [npc, ncls], F32, kind="ExternalOutput")

    rg = [list(range(cores))]

    with tile.TileContext(nc) as tc:
        nc.gpsimd.load_library(mlp_lib)
        with (
            tc.tile_pool(name="const", bufs=1) as cp,
            tc.tile_pool(name="sbw", bufs=2) as sbw,       # window tiles
            tc.tile_pool(name="sbm", bufs=2) as sbm,       # MLP scratch
            tc.tile_pool(name="psq", bufs=2, space="PSUM") as psq,
            tc.tile_pool(name="psa", bufs=2, space="PSUM") as psa,
            tc.tile_pool(name="psu", bufs=1, space="PSUM") as psu,
            tc.tile_pool(name="pst", bufs=2, space="PSUM") as pst,
            tc.tile_pool(name="pso", bufs=1, space="PSUM") as pso,
            tc.tile_pool(name="dram", bufs=2, space="DRAM") as dp,
            tc.tile_pool(name="dram1", bufs=1, space="DRAM") as dp1,
        ):
            # ---------------- constants / weights to SBUF ----------------
            def load(name, dram_ap, shape, rearr=None, dt_=F32, **kw):
                t = cp.tile(shape, dt_, tag=name)
                src = dram_ap if rearr is None else dram_ap.rearrange(rearr, **kw)
                nc.sync.dma_start(t[:], src)
                return t

            wnode_s = load("wnode", wnode_d[:, :], [fin + 1, hid])
            wenc_s = load("wenc", wenc_d[:, :], [fe + 1, hid], dt_=F16)
            w1_s = load("w1", w1_d[:, :, :], [hid, L, H2], "l k n -> k l n",
                        dt_=F16)
            w2_s = load("w2", w2_d[:, :, :], [128, L, 2, hid],
                        "l (h k) n -> k l h n", h=2, dt_=F16)
            b1_s = load("b1", b1_d[:, :], [1, L * H2])
            b2_s = load("b2", b2_d[:, :], [1, L * hid])
            g1_s = load("g1", g1bc_d[:, :, :], [128, L, H2], "l p n -> p l n")
            bb1_s = load("bb1", bb1bc_d[:, :, :], [128, L, H2], "l p n -> p l n")
            ng_s = load("ng", ngbc_d[:, :, :], [128, L, hid], "l p n -> p l n")
            nb_s = load("nb", nbbc_d[:, :, :], [128, L, hid], "l p n -> p l n")
            tcol_s = load("tcol", tcol_d[:, :], [128, L])
            linw_s = load("linw", linw_d[:, :], [hid, ncls], dt_=F16)
            linb_s = load("linb", linb_d[:, :], [1, ncls])
            iota_s = load("iota", iota_d[:, :], [128, 128])
            ident_s = load("ident", ident_d[:, :], [128, 128])
            dstloc_s = load("dstloc", dstloc_d[:, :], [128, W * T])
            gidx_s = cp.tile([128, W * T * 8], mybir.dt.int16, tag="gidx")
            nc.sync.dma_start(gidx_s[:], gidx_d[:, :])

            eps_col = cp.tile([128, 1], F32, tag="epsc")
            nc.vector.memset(eps_col[:], MSG_EPS)
            sm_col = cp.tile([128, 1], F32, tag="smc")
            nc.vector.memset(sm_col[:], SM_EPS)
            ln_col = cp.tile([128, 1], F32, tag="lnc")
            nc.vector.memset(ln_col[:], LN_EPS)
            ones_row = cp.tile([1, 128], F32, tag="ones")
            nc.vector.memset(ones_row[:], 1.0)

            h_state = dp1.tile([npc, hid], F32)

            # ---------------- helpers ----------------
            def layer_norm_relu(dst, src_ap, nfeat, g_ap, b_ap, sq_tag):
                """dst <- relu(LN(src) * g + b).  src may be PSUM."""
                ssum = sbm.tile([128, 1], F32, tag="lnsum")
                nc.vector.reduce_sum(ssum[:], src_ap, axis=mybir.AxisListType.X)
                mcol = sbm.tile([128, 1], F32, tag="lnm")
                nc.scalar.mul(mcol[:], ssum[:], 1.0 / nfeat)
                xm = sbm.tile([128, nfeat], F32, tag="lnxm" + sq_tag)
                if LN_STAGE == 1:
                    nc.vector.tensor_copy(xm[:], src_ap)
                    nc.scalar.activation(dst, xm[:], AF.Relu)
                    return
                nc.vector.tensor_scalar_sub(xm[:], src_ap, mcol[:])
                if LN_STAGE == 2:
                    nc.scalar.activation(dst, xm[:], AF.Relu)
                    return
                sq = sbm.tile([128, nfeat], F32, tag="lnsq" + sq_tag)
                vsum = sbm.tile([128, 1], F32, tag="lnv")
                nc.vector.tensor_mul(sq[:], xm[:], xm[:])
                nc.vector.reduce_sum(vsum[:], sq[:], axis=mybir.AxisListType.X)
                if LN_STAGE == 3:
                    nc.scalar.activation(dst, sq[:], AF.Relu)
                    return
                # rstd = rsqrt(v/nfeat + eps): Quake seed + 2 Newton steps
                a_t = sbm.tile([128, 1], F32, tag="lnva")
                nc.vector.tensor_scalar(a_t[:], vsum[:], 1.0 / nfeat, LN_EPS,
                                        op0=ALU.mult, op1=ALU.add)
                g_t = sbm.tile([128, 1], F32, tag="lnq1")
                nc.vector.tensor_scalar(g_t[:].bitcast(I32),
                                        a_t[:].bitcast(I32), 1, None,
                                        op0=ALU.arith_shift_right)
                g2_t = sbm.tile([128, 1], F32, tag="lnq2")
                nc.vector.tensor_scalar(g2_t[:].bitcast(I32),
                                        g_t[:].bitcast(I32), -1, 0x5f3759df,
                                        op0=ALU.mult, op1=ALU.add)
                rstd = g2_t
                for _ in range(2):
                    gg = sbm.tile([128, 1], F32, tag="lnq3")
                    nc.vector.tensor_mul(gg[:], rstd[:], rstd[:])
                    nc.vector.tensor_mul(gg[:], gg[:], a_t[:])
                    nc.vector.tensor_scalar(gg[:], gg[:], -0.5, 1.5,
                                            op0=ALU.mult, op1=ALU.add)
                    gn = sbm.tile([128, 1], F32, tag="lnq4")
                    nc.vector.tensor_mul(gn[:], rstd[:], gg[:])
                    rstd = gn
                if LN_STAGE == 4:
                    nc.vector.tensor_scalar_mul(xm[:], xm[:], rstd[:])
                    nc.scalar.activation(dst, xm[:], AF.Relu)
                    return
                y = sbm.tile([128, nfeat], F32, tag="lny" + sq_tag)
                nc.vector.tensor_scalar_mul(y[:], xm[:], rstd[:])
                if NO_INPLACE:
                    y2 = sbm.tile([128, nfeat], F32, tag="lnyy" + sq_tag)
                    nc.vector.tensor_mul(y2[:], y[:], g_ap)
                    y3 = sbm.tile([128, nfeat], F32, tag="lnyz" + sq_tag)
                    nc.vector.tensor_add(y3[:], y2[:], b_ap)
                    nc.scalar.activation(dst, y3[:], AF.Relu)
                else:
                    nc.vector.tensor_mul(y[:], y[:], g_ap)
                    nc.vector.tensor_add(y[:], y[:], b_ap)
                    nc.scalar.activation(dst, y[:], AF.Relu)

            def transpose128(src_ap, tag, dt_=F16):
                """PE transpose [128,128] -> SBUF (cast on copy-out)."""
                pt = pst.tile([128, 128], F32, tag="pt")
                nc.tensor.transpose(pt[:], src_ap, ident_s[:])
                st = sbm.tile([128, 128], dt_, tag=tag)
                nc.vector.tensor_copy(st[:], pt[:])
                return st

            # ---------------- encode phase: h0 = x @ wnode ----------------
            ag = dp.tile([npc, hid], F32, tag="ag")
            for w in range(W):
                base = w * 128
                ws = min(128, npc - base)
                xts = sbm.tile([fin + 1, 128], F32, tag="xts")
                nc.sync.dma_start(xts[:, :ws], xT_d[:, base:base + ws])
                ph = pso.tile([128, hid], F32, tag="po")
                nc.tensor.matmul(ph[:ws, :], xts[:, :ws], wnode_s[:],
                                 start=True, stop=True)
                h0 = sbm.tile([128, hid], F32, tag="h0")
                nc.vector.tensor_copy(h0[:ws, :], ph[:ws, :])
                nc.sync.dma_start(ag[base:base + ws, :], h0[:ws, :])

            table = dp.tile([n_nodes, hid], F32, tag="table",
                            addr_space="Shared")
            nc.gpsimd.collective_compute(
                "AllGather", ALU.bypass, ins=[ag.opt()], outs=[table.opt()],
                replica_groups=rg)

            # ---------------- conv layers ----------------
            NQ = (T + 3) // 4  # quads of <=4 tiles

            for li in range(L):
                ag_next = (dp.tile([npc, hid], F32, tag="ag", name=f"agn{li}")
                           if li < L - 1 else None)
                for w in range(W):
                    base = w * 128
                    ws = min(128, npc - base)
                    jw = w * T

                    # gather h[src] for this window: [128, T, 128]
                    # (split into <=1024-index chunks: the SWDGE gather
                    # ucode wedges the device above ~1024 descriptors)
                    hsrc = sbw.tile([128, T, 128], F32, tag="hsrc")
                    if NO_GATHER:
                        nc.vector.memset(hsrc[:], 0.01)
                    else:
                        for c0 in range(0, T, 8):
                            ct = min(8, T - c0)
                            nc.gpsimd.dma_gather(
                                hsrc[:, c0:c0 + ct, :], table[:, :],
                                gidx_s[:, (w * T + c0) * 8:
                                       (w * T + c0 + ct) * 8],
                                ct * 128, ct * 128, hid)
                    attrs = sbw.tile([fe + 1, T, 128], F16, tag="attrs")
                    nc.sync.dma_start(
                        attrs[:], attrT_d[:, w * T * 128:(w + 1) * T * 128])

                    msg = sbw.tile([128, T, 128], F16, tag="msg")
                    em = sbw.tile([128, 2, T, 128], F16, tag="em", bufs=1)
                    # quads: ea matmul + gathered-h add (identity matmul)
                    for q in range(NQ):
                        q0 = q * 4
                        qs = min(4, T - q0)
                        pq = psq.tile([128, 4, 128], F32, tag="pq")
                        for j in range(qs):
                            nc.tensor.matmul(
                                pq[:, j, :], attrs[:, q0 + j, :], wenc_s[:],
                                start=(j == 0), stop=(j == qs - 1),
                                skip_group_check=True)
                        sc = sbw.tile([128, 4, 128], F32, tag="sc", bufs=3)
                        nc.vector.tensor_add(sc[:, :qs, :], pq[:, :qs, :],
                                             hsrc[:, q0:q0 + qs, :])
                        # msg = relu(ea + h_src + eps)
                        nc.scalar.activation(
                            msg[:, q0:q0 + qs, :], sc[:, :qs, :], AF.Relu,
                            bias=eps_col[:])
                    # e = exp(t * msg) ; me = msg * e
                    nc.scalar.activation(
                        em[:, 0, :, :], msg[:], AF.Exp,
                        scale=tcol_s[:, li:li + 1])
                    me_eng = nc.gpsimd if ME_ENGINE == "gpsimd" else nc.vector
                    me_eng.tensor_tensor(
                        em[:, 1, :, :], msg[:], em[:, 0, :, :], op=ALU.mult)
                    # one-hot S for the whole window
                    S = sbw.tile([128, T, 128], F16, tag="S")
                    if NO_ONEHOT:
                        nc.vector.memset(S[:], 0.0)
                    else:
                        iota_b = iota_s[:].rearrange(
                            "p (o f) -> p o f", o=1).broadcast_to([128, T, 128])
                        dl_b = dstloc_s[:, jw:jw + T].rearrange(
                            "p (t o) -> p t o", o=1).broadcast_to([128, T, 128])
                        nc.vector.tensor_tensor(S[:], iota_b, dl_b,
                                                op=ALU.is_equal)
                    # segment accumulate [sum e | sum me] -> [128, 256] psum
                    acc = psa.tile([128, 2, hid], F32, tag="acc")
                    for g in range(T):
                        nc.tensor.matmul(
                            acc[:, :, :], S[:, g, :], em[:, :, g, :],
                            start=(g == 0), stop=(g == T - 1))

                    # agg = (sum me) / (sum e + eps)
                    sep = sbm.tile([128, hid], F32, tag="sep")
                    nc.vector.tensor_scalar_add(sep[:], acc[:, 0, :], SM_EPS)
                    rcse = sbm.tile([128, hid], F32, tag="rcse")
                    nc.vector.reciprocal_approx_fast(rcse[:], sep[:])
                    z = sbm.tile([128, hid], F32, tag="z")
                    nc.vector.tensor_mul(z[:], acc[:, 1, :], rcse[:])
                    # z += conv input rows (this core's shard of table source)
                    zin = sbm.tile([128, hid], F32, tag="zin")
                    nc.sync.dma_start(zin[:ws, :], ag[base:base + ws, :])
                    if NO_INPLACE:
                        z2t = sbm.tile([128, hid], F32, tag="z2t")
                        nc.vector.tensor_add(z2t[:], z[:], zin[:])
                        z = z2t
                    else:
                        nc.vector.tensor_add(z[:], z[:], zin[:])

                    if SKIP_MLP:
                        nc.sync.dma_start(out_d[base:base + ws, :],
                                          z[:ws, :ncls])
                        continue

                    # ---- MLP: relu(LN(z@w1+b1))@w2+b2 ----
                    zT = transpose128(z[:], "zT")
                    if MLP_STAGE == 1:
                        nc.sync.dma_start(out_d[base:base + ws, :],
                                          zT[:ws, :ncls])
                        continue
                    pu = psu.tile([128, H2], F32, tag="pu")
                    nc.tensor.matmul(pu[:], zT[:], w1_s[:, li, :],
                                     start=True, stop=False,
                                     skip_group_check=True)
                    nc.tensor.matmul(pu[:], ones_row[:],
                                     b1_s[:, li * H2:(li + 1) * H2],
                                     start=False, stop=True,
                                     skip_group_check=True)
                    if MLP_STAGE == 2:
                        uu = sbm.tile([128, ncls], F32, tag="uu")
                        nc.vector.tensor_copy(uu[:], pu[:, :ncls])
                        nc.sync.dma_start(out_d[base:base + ws, :],
                                          uu[:ws, :])
                        continue
                    r = sbm.tile([128, H2], F32, tag="r")
                    layer_norm_relu(r[:], pu[:], H2, g1_s[:, li, :],
                                    bb1_s[:, li, :], "a")
                    if MLP_STAGE == 3:
                        nc.sync.dma_start(out_d[base:base + ws, :],
                                          r[:ws, :ncls])
                        continue
                    rT0 = transpose128(r[:, 0:128], "rT0")
                    rT1 = transpose128(r[:, 128:256], "rT1")
                    po = pso.tile([128, hid], F32, tag="po")
                    nc.tensor.matmul(po[:], rT0[:], w2_s[:, li, 0, :],
                                     start=True, stop=False,
                                     skip_group_check=True)
                    nc.tensor.matmul(po[:], rT1[:], w2_s[:, li, 1, :],
                                     start=False, stop=False,
                                     skip_group_check=True)
                    nc.tensor.matmul(po[:], ones_row[:],
                                     b2_s[:, li * hid:(li + 1) * hid],
                                     start=False, stop=True,
                                     skip_group_check=True)

                    if MLP_STAGE == 4:
                        oo = sbm.tile([128, ncls], F32, tag="oo")
                        nc.vector.tensor_copy(oo[:], po[:, :ncls])
                        nc.sync.dma_start(out_d[base:base + ws, :],
                                          oo[:ws, :])
                        continue
                    # ---- layer epilogue ----
                    hcur = sbm.tile([128, hid], F32, tag="hcur")
                    if li == 0:
                        nc.vector.tensor_copy(hcur[:], po[:])
                    else:
                        hprev = sbm.tile([128, hid], F32, tag="hprev")
                        nc.sync.dma_start(hprev[:ws, :],
                                          h_state[base:base + ws, :])
                        nc.vector.tensor_add(hcur[:], po[:], hprev[:])
                    if li < L - 1:
                        nc.sync.dma_start(h_state[base:base + ws, :],
                                          hcur[:ws, :])
                        # z for next layer: relu(LN(h; norm[li+1]))
                        znext = sbm.tile([128, hid], F32, tag="znext")
                        layer_norm_relu(znext[:], hcur[:], hid,
                                        ng_s[:, li + 1, :], nb_s[:, li + 1, :],
                                        "b")
                        nc.sync.dma_start(ag_next[base:base + ws, :],
                                          znext[:ws, :])
                    else:
                        # final: relu(LN(h; norm[0])) @ lin_w + lin_b
                        fin_t = sbm.tile([128, hid], F32, tag="fin")
                        layer_norm_relu(fin_t[:], hcur[:], hid,
                                        ng_s[:, 0, :], nb_s[:, 0, :], "b")
                        finT = transpose128(fin_t[:], "finT")
                        pc = pso.tile([128, ncls], F32, tag="po")
                        nc.tensor.matmul(pc[:], finT[:], linw_s[:],
                                         start=True, stop=False,
                                         skip_group_check=True)
                        nc.tensor.matmul(pc[:], ones_row[:], linb_s[:],
                                         start=False, stop=True,
                                         skip_group_check=True)
                        ow = sbm.tile([128, ncls], F32, tag="ow")
                        nc.vector.tensor_copy(ow[:], pc[:])
                        nc.sync.dma_start(out_d[base:base + ws, :],
                                          ow[:ws, :])

                if li < L - 1:
                    table = dp.tile([n_nodes, hid], F32, tag="table",
                                    addr_space="Shared")
                    nc.gpsimd.collective_compute(
                        "AllGather", ALU.bypass, ins=[ag_next.opt()],
                        outs=[table.opt()], replica_groups=rg)
                    ag = ag_next

    nc.compile()
    return nc


# --------------------------------------------------------------------------
# entry point
# --------------------------------------------------------------------------

_CACHE = {}


def _get_program(T, W, npc):
    key = (T, W, npc)
    if key not in _CACHE:
        _CACHE[key] = build_program(T, W, npc)
    return _CACHE[key]


def _install_ntff_hook():
    """Bridge trn_agent_boot's ctypes NTFF profiler into antenv.axon_hooks
    (absent from this image) so run_bass_kernel_spmd(trace=True) works."""
    import types

    if "antenv.axon_hooks" in sys.modules:
        return
    try:
        sys.path.insert(0, "/root/.axon_site")
        from trn_agent_boot.trn_boot import _ntff_profile_via_ctypes

        hook = _ntff_profile_via_ctypes("/opt/axon/libaxon_pjrt.so")
    except Exception:
        hook = None
    m = types.ModuleType("antenv.axon_hooks")
    state = {"hook": hook}
    m.get_axon_ntff_profile_hook = lambda: state["hook"]
    m.set_axon_ntff_profile_hook = lambda h: state.update(hook=h)
    sys.modules["antenv.axon_hooks"] = m
    import antenv

    antenv.axon_hooks = m


def run(inputs, trace=False):
    if trace:
        _install_ntff_hook()
    T, W, npc, in_maps = _prep_inputs(inputs)
    nc = _get_program(T, W, npc)
    res = run_bass_kernel_spmd(nc, in_maps, list(range(CORES)), trace=trace)
    out = np.concatenate([res.results[c]["out"] for c in range(CORES)], axis=0)
    return out, res


def kernel(**inputs) -> np.ndarray:
    out, _ = run(inputs, trace=False)
    return out


# revision 20
# speedup vs baseline: 2.1674x; 1.2927x over previous
"""CGCNN / GENConv GNN message-passing kernel for 8 Trainium2 NeuronCores.

Strategy (dst-sharded edge parallel):
  - Host sorts edges by dst and shards them by dst node range: core k owns
    nodes [k*3750, (k+1)*3750) and every edge pointing into that range.
    Segment softmax/sums therefore never cross cores.
  - Each layer: every core gathers h[src] for its edges from a replicated
    node table in its local DRAM (dma_gather, 512B rows), computes
    msg = relu(h_src + ea) + eps, e = exp(t*msg), me = msg*e, and
    segment-reduces [sum e | sum me] per 128-node window with a one-hot
    matmul accumulated in PSUM.  alpha-normalization folds into a single
    per-node divide: agg = (sum me) / (sum e + eps)  (exactly equal to the
    reference softmax aggregation up to ~1e-16: the max-subtraction in the
    reference cancels algebraically and logits here are O(1)).
  - Node MLP / LayerNorm runs data-parallel on the core's own node shard.
  - The updated table (conv input of the next layer) is AllGather'ed
    across the 8 cores (~1.9MB per rank).

kernel(**inputs) takes the FULL inputs and returns the FULL [30000, 10]
output; sharding + compilation happen inside (compiled program is cached).
"""

import os
import sys

sys.path.insert(0, "/opt/trn_rl_repo")

ME_ENGINE = os.environ.get("K_ME_ENGINE", "vector")  # gpsimd | vector
NO_GATHER = os.environ.get("K_NO_GATHER", "0") == "1"
NO_ONEHOT = os.environ.get("K_NO_ONEHOT", "0") == "1"
SKIP_MLP = os.environ.get("K_SKIP_MLP", "0") == "1"
NO_INPLACE = os.environ.get("K_NO_INPLACE", "0") == "1"
MLP_STAGE = int(os.environ.get("K_MLP_STAGE", "9"))
LN_STAGE = int(os.environ.get("K_LN_STAGE", "9"))

import numpy as np

import concourse.bacc as bacc
import concourse.bass as bass
import concourse.mybir as mybir
import concourse.tile as tile
from concourse.bass_utils import run_bass_kernel_spmd
from concourse.library_config import mlp as mlp_lib

F32 = mybir.dt.float32
F16 = mybir.dt.float16
F32R = mybir.dt.float32r
I32 = mybir.dt.int32
AF = mybir.ActivationFunctionType
ALU = mybir.AluOpType

MSG_EPS = 1e-7
SM_EPS = 1e-16
LN_EPS = 1e-5

# problem dims (hardcoded per harness contract)
N_NODES = 30000
N_EDGES = 480000
F_IN = 64
F_EDGE = 16
HID = 128
N_LAYERS = 3
N_CLASSES = 10
CORES = 8


# --------------------------------------------------------------------------
# host-side sharding / packing
# --------------------------------------------------------------------------

def _prep_edges(edge_index, edge_attr, n_nodes, cores, npc, win):
    """Sort edges by dst, shard by dst range, pack per (core, window, tile).

    win = nodes per window (128).  Returns (T, per-core dict arrays).
    """
    src = edge_index[0].astype(np.int64)
    dst = edge_index[1].astype(np.int64)
    order = np.argsort(dst, kind="stable")
    src = src[order]
    dst = dst[order]
    attr = edge_attr[order]

    W = (npc + win - 1) // win  # windows per core
    # window id of each edge globally: core * W + (local node // win)
    core_of = dst // npc
    wloc = (dst - core_of * npc) // win
    gwin = core_of * W + wloc
    # edges are sorted by dst so gwin is non-decreasing
    counts = np.bincount(gwin, minlength=cores * W)
    T = int(np.ceil(counts.max() / 128.0))
    T = max(T, 1)
    epw = T * 128  # padded edges per window
    EPAD = W * epw

    starts = np.zeros(cores * W + 1, np.int64)
    np.cumsum(counts, out=starts[1:])

    fe = attr.shape[1]
    src_pad = np.zeros((cores, W, epw), np.int64)
    dstloc_pad = np.full((cores, W, epw), -1.0, np.float32)
    attr_pad = np.zeros((cores, W, epw, fe), np.float32)
    for c in range(cores):
        for w in range(W):
            g = c * W + w
            s, e = starts[g], starts[g + 1]
            n = e - s
            src_pad[c, w, :n] = src[s:e]
            dstloc_pad[c, w, :n] = (dst[s:e] - (c * npc + w * win)).astype(
                np.float32
            )
            attr_pad[c, w, :n, :] = attr[s:e]

    # gather index layout: [128, W*T*8] int16, idx i of window w at
    # partition i%16 (replicated x8), column w*T*8 + i//16
    sp = src_pad.reshape(cores, W, T * 8, 16)
    gidx16 = np.transpose(sp, (0, 3, 1, 2)).reshape(cores, 16, W * T * 8)
    gidx = np.tile(gidx16, (1, 8, 1)).astype(np.int16)

    # dstloc: [128, W*T] f32, tile j=w*T+g column, partition = edge offset
    dl = dstloc_pad.reshape(cores, W, T, 128)
    dstloc = np.ascontiguousarray(
        np.transpose(dl, (0, 3, 1, 2)).reshape(cores, 128, W * T)
    )

    # attrT: [17, EPAD]: rows 0..15 features (transposed), row 16 = ones
    ap = attr_pad.reshape(cores, EPAD, fe)
    attrT = np.empty((cores, fe + 1, EPAD), np.float16)
    attrT[:, :fe, :] = np.transpose(ap, (0, 2, 1))
    attrT[:, fe, :] = 1.0
    return T, W, gidx, dstloc, np.ascontiguousarray(attrT)


def _prep_inputs(inputs, cores=CORES):
    """Build the 8 per-core input maps (and shared weight arrays)."""
    x = np.asarray(inputs["x"], np.float32)
    edge_attr = np.asarray(inputs["edge_attr"], np.float32)
    edge_index = np.asarray(inputs["edge_index"])
    n_nodes, fin = x.shape
    npc = n_nodes // cores
    win = 128

    T, W, gidx, dstloc, attrT = _prep_edges(
        edge_index, edge_attr, n_nodes, cores, npc, win
    )

    L = int(np.asarray(inputs["t"]).shape[0])
    hid = np.asarray(inputs["node_enc_w"]).shape[1]

    # xT per core: [fin+1, npc] with ones row
    xs = x.reshape(cores, npc, fin)
    xT = np.empty((cores, fin + 1, npc), np.float32)
    xT[:, :fin, :] = np.transpose(xs, (0, 2, 1))
    xT[:, fin, :] = 1.0

    wnode = np.concatenate(
        [np.asarray(inputs["node_enc_w"], np.float32),
         np.asarray(inputs["node_enc_b"], np.float32)[None, :]], 0
    )
    wenc = np.concatenate(
        [np.asarray(inputs["edge_enc_w"], np.float32),
         np.asarray(inputs["edge_enc_b"], np.float32)[None, :]], 0
    )
    w1 = np.ascontiguousarray(inputs["mlp1_w"], np.float32)      # [L,H,2H]
    w2 = np.ascontiguousarray(inputs["mlp2_w"], np.float32)      # [L,2H,H]
    b1 = np.ascontiguousarray(
        np.asarray(inputs["mlp1_b"], np.float32).reshape(1, -1))  # [1,L*2H]
    b2 = np.ascontiguousarray(
        np.asarray(inputs["mlp2_b"], np.float32).reshape(1, -1))  # [1,L*H]
    g1bc = np.ascontiguousarray(
        np.broadcast_to(np.asarray(inputs["mlp_ln_g"], np.float32)[:, None, :],
                        (L, 128, 2 * hid)))
    bb1bc = np.ascontiguousarray(
        np.broadcast_to(np.asarray(inputs["mlp_ln_b"], np.float32)[:, None, :],
                        (L, 128, 2 * hid)))
    ngbc = np.ascontiguousarray(
        np.broadcast_to(np.asarray(inputs["norm_g"], np.float32)[:, None, :],
                        (L, 128, hid)))
    nbbc = np.ascontiguousarray(
        np.broadcast_to(np.asarray(inputs["norm_b"], np.float32)[:, None, :],
                        (L, 128, hid)))
    tcol = np.ascontiguousarray(
        np.broadcast_to(np.asarray(inputs["t"], np.float32)[None, :], (128, L)))
    linw = np.ascontiguousarray(inputs["lin_w"], np.float32)
    linb = np.ascontiguousarray(
        np.asarray(inputs["lin_b"], np.float32)[None, :])
    iota = np.ascontiguousarray(
        np.broadcast_to(np.arange(128, dtype=np.float32)[None, :], (128, 128)))
    ident = np.eye(128, dtype=np.float32)

    wench = wenc.astype(np.float16)
    w1h = w1.astype(np.float16)
    w2h = w2.astype(np.float16)
    linwh = linw.astype(np.float16)
    shared = dict(wnode=wnode, wench=wench, w1h=w1h, w2h=w2h, b1=b1, b2=b2,
                  g1bc=g1bc, bb1bc=bb1bc, ngbc=ngbc, nbbc=nbbc, tcol=tcol,
                  linwh=linwh, linb=linb, iota=iota, ident=ident)
    in_maps = []
    for c in range(cores):
        m = dict(shared)
        m["xT"] = np.ascontiguousarray(xT[c])
        m["attrT"] = attrT[c]
        m["gidx"] = np.ascontiguousarray(gidx[c])
        m["dstloc"] = dstloc[c]
        in_maps.append(m)
    return T, W, npc, in_maps


# --------------------------------------------------------------------------
# device program
# --------------------------------------------------------------------------

def build_program(T, W, npc, n_nodes=N_NODES, cores=CORES, fin=F_IN,
                  fe=F_EDGE, hid=HID, L=N_LAYERS, ncls=N_CLASSES):
    EPAD = W * T * 128
    H2 = 2 * hid
    nc = bacc.Bacc("TRN2", target_bir_lowering=False, debug=False,
                   num_devices=cores, dynamic_dma_scratch_size=65536,
                   num_swdge_queues=2)

    d = nc.dram_tensor
    xT_d = d("xT", [fin + 1, npc], F32, kind="ExternalInput")
    attrT_d = d("attrT", [fe + 1, EPAD], F16, kind="ExternalInput")
    gidx_d = d("gidx", [128, W * T * 8], mybir.dt.int16, kind="ExternalInput")
    dstloc_d = d("dstloc", [128, W * T], F32, kind="ExternalInput")
    wnode_d = d("wnode", [fin + 1, hid], F32, kind="ExternalInput")
    wenc_d = d("wench", [fe + 1, hid], F16, kind="ExternalInput")
    w1_d = d("w1h", [L, hid, H2], F16, kind="ExternalInput")
    w2_d = d("w2h", [L, H2, hid], F16, kind="ExternalInput")
    b1_d = d("b1", [1, L * H2], F32, kind="ExternalInput")
    b2_d = d("b2", [1, L * hid], F32, kind="ExternalInput")
    g1bc_d = d("g1bc", [L, 128, H2], F32, kind="ExternalInput")
    bb1bc_d = d("bb1bc", [L, 128, H2], F32, kind="ExternalInput")
    ngbc_d = d("ngbc", [L, 128, hid], F32, kind="ExternalInput")
    nbbc_d = d("nbbc", [L, 128, hid], F32, kind="ExternalInput")
    tcol_d = d("tcol", [128, L], F32, kind="ExternalInput")
    linw_d = d("linwh", [hid, ncls], F16, kind="ExternalInput")
    linb_d = d("linb", [1, ncls], F32, kind="ExternalInput")
    iota_d = d("iota", [128, 128], F32, kind="ExternalInput")
    ident_d = d("ident", [128, 128], F32, kind="ExternalInput")
    out_d = d("out", [npc, ncls], F32, kind="ExternalOutput")

    rg = [list(range(cores))]

    with tile.TileContext(nc) as tc:
        nc.gpsimd.load_library(mlp_lib)
        with (
            tc.tile_pool(name="const", bufs=1) as cp,
            tc.tile_pool(name="sbw", bufs=2) as sbw,       # window tiles
            tc.tile_pool(name="sbm", bufs=2) as sbm,       # MLP scratch
            tc.tile_pool(name="psq", bufs=2, space="PSUM") as psq,
            tc.tile_pool(name="psa", bufs=2, space="PSUM") as psa,
            tc.tile_pool(name="psu", bufs=1, space="PSUM") as psu,
            tc.tile_pool(name="pst", bufs=2, space="PSUM") as pst,
            tc.tile_pool(name="pso", bufs=1, space="PSUM") as pso,
            tc.tile_pool(name="dram", bufs=2, space="DRAM") as dp,
            tc.tile_pool(name="dram1", bufs=1, space="DRAM") as dp1,
        ):
            # ---------------- constants / weights to SBUF ----------------
            def load(name, dram_ap, shape, rearr=None, dt_=F32, **kw):
                t = cp.tile(shape, dt_, tag=name)
                src = dram_ap if rearr is None else dram_ap.rearrange(rearr, **kw)
                nc.sync.dma_start(t[:], src)
                return t

            wnode_s = load("wnode", wnode_d[:, :], [fin + 1, hid])
            wenc_s = load("wenc", wenc_d[:, :], [fe + 1, hid], dt_=F16)
            w1_s = load("w1", w1_d[:, :, :], [hid, L, H2], "l k n -> k l n",
                        dt_=F16)
            w2_s = load("w2", w2_d[:, :, :], [128, L, 2, hid],
                        "l (h k) n -> k l h n", h=2, dt_=F16)
            b1_s = load("b1", b1_d[:, :], [1, L * H2])
            b2_s = load("b2", b2_d[:, :], [1, L * hid])
            g1_s = load("g1", g1bc_d[:, :, :], [128, L, H2], "l p n -> p l n")
            bb1_s = load("bb1", bb1bc_d[:, :, :], [128, L, H2], "l p n -> p l n")
            ng_s = load("ng", ngbc_d[:, :, :], [128, L, hid], "l p n -> p l n")
            nb_s = load("nb", nbbc_d[:, :, :], [128, L, hid], "l p n -> p l n")
            tcol_s = load("tcol", tcol_d[:, :], [128, L])
            linw_s = load("linw", linw_d[:, :], [hid, ncls], dt_=F16)
            linb_s = load("linb", linb_d[:, :], [1, ncls])
            iota_s = load("iota", iota_d[:, :], [128, 128])
            ident_s = load("ident", ident_d[:, :], [128, 128])
            dstloc_s = load("dstloc", dstloc_d[:, :], [128, W * T])
            gidx_s = cp.tile([128, W * T * 8], mybir.dt.int16, tag="gidx")
            nc.sync.dma_start(gidx_s[:], gidx_d[:, :])

            eps_col = cp.tile([128, 1], F32, tag="epsc")
            nc.vector.memset(eps_col[:], MSG_EPS)
            sm_col = cp.tile([128, 1], F32, tag="smc")
            nc.vector.memset(sm_col[:], SM_EPS)
            ln_col = cp.tile([128, 1], F32, tag="lnc")
            nc.vector.memset(ln_col[:], LN_EPS)
            ones_row = cp.tile([1, 128], F32, tag="ones")
            nc.vector.memset(ones_row[:], 1.0)

            h_state = dp1.tile([npc, hid], F32)

            # ---------------- helpers ----------------
            def layer_norm_relu(dst, src_ap, nfeat, g_ap, b_ap, sq_tag):
                """dst <- relu(LN(src) * g + b).  src may be PSUM."""
                ssum = sbm.tile([128, 1], F32, tag="lnsum")
                nc.vector.reduce_sum(ssum[:], src_ap, axis=mybir.AxisListType.X)
                mcol = sbm.tile([128, 1], F32, tag="lnm")
                nc.scalar.mul(mcol[:], ssum[:], 1.0 / nfeat)
                xm = sbm.tile([128, nfeat], F32, tag="lnxm" + sq_tag)
                mb = mcol[:].rearrange("p (o f) -> p o f", o=1).broadcast_to(
                    [128, 1, nfeat])
                nc.vector.tensor_tensor(
                    xm[:].rearrange("p (o f) -> p o f", o=1), src_ap.rearrange(
                        "p (o f) -> p o f", o=1), mb, op=ALU.subtract)
                if LN_STAGE == 2:
                    nc.scalar.activation(dst, xm[:], AF.Relu)
                    return
                sq = sbm.tile([128, nfeat], F32, tag="lnsq" + sq_tag)
                vsum = sbm.tile([128, 1], F32, tag="lnv")
                nc.vector.tensor_mul(sq[:], xm[:], xm[:])
                nc.vector.reduce_sum(vsum[:], sq[:], axis=mybir.AxisListType.X)
                if LN_STAGE == 3:
                    nc.scalar.activation(dst, sq[:], AF.Relu)
                    return
                # rstd = rsqrt(v/nfeat + eps): Quake seed + 2 Newton steps
                a_t = sbm.tile([128, 1], F32, tag="lnva")
                nc.vector.tensor_scalar(a_t[:], vsum[:], 1.0 / nfeat, LN_EPS,
                                        op0=ALU.mult, op1=ALU.add)
                g_t = sbm.tile([128, 1], F32, tag="lnq1")
                nc.vector.tensor_scalar(g_t[:].bitcast(I32),
                                        a_t[:].bitcast(I32), 1, None,
                                        op0=ALU.arith_shift_right)
                g2_t = sbm.tile([128, 1], F32, tag="lnq2")
                nc.vector.tensor_scalar(g2_t[:].bitcast(I32),
                                        g_t[:].bitcast(I32), -1, 0x5f3759df,
                                        op0=ALU.mult, op1=ALU.add)
                rstd = g2_t
                for _ in range(2):
                    gg = sbm.tile([128, 1], F32, tag="lnq3")
                    nc.vector.tensor_mul(gg[:], rstd[:], rstd[:])
                    nc.vector.tensor_mul(gg[:], gg[:], a_t[:])
                    nc.vector.tensor_scalar(gg[:], gg[:], -0.5, 1.5,
                                            op0=ALU.mult, op1=ALU.add)
                    gn = sbm.tile([128, 1], F32, tag="lnq4")
                    nc.vector.tensor_mul(gn[:], rstd[:], gg[:])
                    rstd = gn
                if LN_STAGE == 4:
                    nc.vector.tensor_scalar_mul(xm[:], xm[:], rstd[:])
                    nc.scalar.activation(dst, xm[:], AF.Relu)
                    return
                y = sbm.tile([128, nfeat], F32, tag="lny" + sq_tag)
                rb = rstd[:].rearrange("p (o f) -> p o f", o=1).broadcast_to(
                    [128, 1, nfeat])
                nc.vector.tensor_tensor(
                    y[:].rearrange("p (o f) -> p o f", o=1),
                    xm[:].rearrange("p (o f) -> p o f", o=1), rb, op=ALU.mult)
                if NO_INPLACE:
                    y2 = sbm.tile([128, nfeat], F32, tag="lnyy" + sq_tag)
                    nc.vector.tensor_mul(y2[:], y[:], g_ap)
                    y3 = sbm.tile([128, nfeat], F32, tag="lnyz" + sq_tag)
                    nc.vector.tensor_add(y3[:], y2[:], b_ap)
                    nc.scalar.activation(dst, y3[:], AF.Relu)
                else:
                    nc.vector.tensor_mul(y[:], y[:], g_ap)
                    nc.vector.tensor_add(y[:], y[:], b_ap)
                    nc.scalar.activation(dst, y[:], AF.Relu)

            def transpose128(src_ap, tag, dt_=F16):
                """PE transpose [128,128] -> SBUF (cast on copy-out)."""
                pt = pst.tile([128, 128], F32, tag="pt")
                nc.tensor.transpose(pt[:], src_ap, ident_s[:])
                st = sbm.tile([128, 128], dt_, tag=tag)
                nc.vector.tensor_copy(st[:], pt[:])
                return st

            # ---------------- encode phase: h0 = x @ wnode ----------------
            ag = dp.tile([npc, hid], F32, tag="ag")
            for w in range(W):
                base = w * 128
                ws = min(128, npc - base)
                xts = sbm.tile([fin + 1, 128], F32, tag="xts")
                nc.sync.dma_start(xts[:, :ws], xT_d[:, base:base + ws])
                ph = pso.tile([128, hid], F32, tag="po")
                nc.tensor.matmul(ph[:ws, :], xts[:, :ws], wnode_s[:],
                                 start=True, stop=True)
                h0 = sbm.tile([128, hid], F32, tag="h0")
                nc.vector.tensor_copy(h0[:ws, :], ph[:ws, :])
                nc.sync.dma_start(ag[base:base + ws, :], h0[:ws, :])

            table = dp.tile([n_nodes, hid], F32, tag="table",
                            addr_space="Shared")
            nc.gpsimd.collective_compute(
                "AllGather", ALU.bypass, ins=[ag.opt()], outs=[table.opt()],
                replica_groups=rg)

            # ---------------- conv layers ----------------
            NQ = (T + 3) // 4  # quads of <=4 tiles

            for li in range(L):
                ag_next = (dp.tile([npc, hid], F32, tag="ag", name=f"agn{li}")
                           if li < L - 1 else None)
                for w in range(W):
                    base = w * 128
                    ws = min(128, npc - base)
                    jw = w * T

                    # gather h[src] for this window: [128, T, 128]
                    # (split into <=1024-index chunks: the SWDGE gather
                    # ucode wedges the device above ~1024 descriptors)
                    hsrc = sbw.tile([128, T, 128], F32, tag="hsrc")
                    if NO_GATHER:
                        nc.vector.memset(hsrc[:], 0.01)
                    else:
                        for qi, c0 in enumerate(range(0, T, 8)):
                            ct = min(8, T - c0)
                            nc.gpsimd.dma_gather(
                                hsrc[:, c0:c0 + ct, :], table[:, :],
                                gidx_s[:, (w * T + c0) * 8:
                                       (w * T + c0 + ct) * 8],
                                ct * 128, ct * 128, hid,
                                queue_num=qi % 2)
                    attrs = sbw.tile([fe + 1, T, 128], F16, tag="attrs")
                    nc.sync.dma_start(
                        attrs[:], attrT_d[:, w * T * 128:(w + 1) * T * 128])

                    msg = sbw.tile([128, T, 128], F16, tag="msg")
                    em = sbw.tile([128, 2, T, 128], F16, tag="em", bufs=1)
                    # quads: ea matmul + gathered-h add (identity matmul)
                    for q in range(NQ):
                        q0 = q * 4
                        qs = min(4, T - q0)
                        pq = psq.tile([128, 4, 128], F32, tag="pq")
                        for j in range(qs):
                            nc.tensor.matmul(
                                pq[:, j, :], attrs[:, q0 + j, :], wenc_s[:],
                                start=(j == 0), stop=(j == qs - 1),
                                skip_group_check=True)
                        sc = sbw.tile([128, 4, 128], F32, tag="sc", bufs=3)
                        nc.vector.tensor_add(sc[:, :qs, :], pq[:, :qs, :],
                                             hsrc[:, q0:q0 + qs, :])
                        # msg = relu(ea + h_src + eps)
                        nc.scalar.activation(
                            msg[:, q0:q0 + qs, :], sc[:, :qs, :], AF.Relu,
                            bias=eps_col[:])
                    # e = exp(t * msg) ; me = msg * e
                    nc.scalar.activation(
                        em[:, 0, :, :], msg[:], AF.Exp,
                        scale=tcol_s[:, li:li + 1])
                    me_eng = nc.gpsimd if ME_ENGINE == "gpsimd" else nc.vector
                    me_eng.tensor_tensor(
                        em[:, 1, :, :], msg[:], em[:, 0, :, :], op=ALU.mult)
                    # one-hot S for the whole window
                    S = sbw.tile([128, T, 128], F16, tag="S")
                    if NO_ONEHOT:
                        nc.vector.memset(S[:], 0.0)
                    else:
                        iota_b = iota_s[:].rearrange(
                            "p (o f) -> p o f", o=1).broadcast_to([128, T, 128])
                        dl_b = dstloc_s[:, jw:jw + T].rearrange(
                            "p (t o) -> p t o", o=1).broadcast_to([128, T, 128])
                        nc.vector.tensor_tensor(S[:], iota_b, dl_b,
                                                op=ALU.is_equal)
                    # segment accumulate [sum e | sum me] -> [128, 256] psum
                    acc = psa.tile([128, 2, hid], F32, tag="acc")
                    for g in range(T):
                        nc.tensor.matmul(
                            acc[:, :, :], S[:, g, :], em[:, :, g, :],
                            start=(g == 0), stop=(g == T - 1))

                    # agg = (sum me) / (sum e + eps)
                    sep = sbm.tile([128, hid], F32, tag="sep")
                    nc.vector.tensor_scalar_add(sep[:], acc[:, 0, :], SM_EPS)
                    rcse = sbm.tile([128, hid], F32, tag="rcse")
                    nc.vector.reciprocal_approx_fast(rcse[:], sep[:])
                    z = sbm.tile([128, hid], F32, tag="z")
                    nc.vector.tensor_mul(z[:], acc[:, 1, :], rcse[:])
                    # z += conv input rows (this core's shard of table source)
                    zin = sbm.tile([128, hid], F32, tag="zin")
                    nc.sync.dma_start(zin[:ws, :], ag[base:base + ws, :])
                    if NO_INPLACE:
                        z2t = sbm.tile([128, hid], F32, tag="z2t")
                        nc.vector.tensor_add(z2t[:], z[:], zin[:])
                        z = z2t
                    else:
                        nc.vector.tensor_add(z[:], z[:], zin[:])

                    if SKIP_MLP:
                        nc.sync.dma_start(out_d[base:base + ws, :],
                                          z[:ws, :ncls])
                        continue

                    # ---- MLP: relu(LN(z@w1+b1))@w2+b2 ----
                    zT = transpose128(z[:], "zT")
                    if MLP_STAGE == 1:
                        nc.sync.dma_start(out_d[base:base + ws, :],
                                          zT[:ws, :ncls])
                        continue
                    pu = psu.tile([128, H2], F32, tag="pu")
                    nc.tensor.matmul(pu[:], zT[:], w1_s[:, li, :],
                                     start=True, stop=False,
                                     skip_group_check=True)
                    nc.tensor.matmul(pu[:], ones_row[:],
                                     b1_s[:, li * H2:(li + 1) * H2],
                                     start=False, stop=True,
                                     skip_group_check=True)
                    if MLP_STAGE == 2:
                        uu = sbm.tile([128, ncls], F32, tag="uu")
                        nc.vector.tensor_copy(uu[:], pu[:, :ncls])
                        nc.sync.dma_start(out_d[base:base + ws, :],
                                          uu[:ws, :])
                        continue
                    r = sbm.tile([128, H2], F32, tag="r")
                    layer_norm_relu(r[:], pu[:], H2, g1_s[:, li, :],
                                    bb1_s[:, li, :], "a")
                    if MLP_STAGE == 3:
                        nc.sync.dma_start(out_d[base:base + ws, :],
                                          r[:ws, :ncls])
                        continue
                    rT0 = transpose128(r[:, 0:128], "rT0")
                    rT1 = transpose128(r[:, 128:256], "rT1")
                    po = pso.tile([128, hid], F32, tag="po")
                    nc.tensor.matmul(po[:], rT0[:], w2_s[:, li, 0, :],
                                     start=True, stop=False,
                                     skip_group_check=True)
                    nc.tensor.matmul(po[:], rT1[:], w2_s[:, li, 1, :],
                                     start=False, stop=False,
                                     skip_group_check=True)
                    nc.tensor.matmul(po[:], ones_row[:],
                                     b2_s[:, li * hid:(li + 1) * hid],
                                     start=False, stop=True,
                                     skip_group_check=True)

                    if MLP_STAGE == 4:
                        oo = sbm.tile([128, ncls], F32, tag="oo")
                        nc.vector.tensor_copy(oo[:], po[:, :ncls])
                        nc.sync.dma_start(out_d[base:base + ws, :],
                                          oo[:ws, :])
                        continue
                    # ---- layer epilogue ----
                    hcur = sbm.tile([128, hid], F32, tag="hcur")
                    if li == 0:
                        nc.vector.tensor_copy(hcur[:], po[:])
                    else:
                        hprev = sbm.tile([128, hid], F32, tag="hprev")
                        nc.sync.dma_start(hprev[:ws, :],
                                          h_state[base:base + ws, :])
                        nc.vector.tensor_add(hcur[:], po[:], hprev[:])
                    if li < L - 1:
                        nc.sync.dma_start(h_state[base:base + ws, :],
                                          hcur[:ws, :])
                        # z for next layer: relu(LN(h; norm[li+1]))
                        znext = sbm.tile([128, hid], F32, tag="znext")
                        layer_norm_relu(znext[:], hcur[:], hid,
                                        ng_s[:, li + 1, :], nb_s[:, li + 1, :],
                                        "b")
                        nc.sync.dma_start(ag_next[base:base + ws, :],
                                          znext[:ws, :])
                    else:
                        # final: relu(LN(h; norm[0])) @ lin_w + lin_b
                        fin_t = sbm.tile([128, hid], F32, tag="fin")
                        layer_norm_relu(fin_t[:], hcur[:], hid,
                                        ng_s[:, 0, :], nb_s[:, 0, :], "b")
                        finT = transpose128(fin_t[:], "finT")
                        pc = pso.tile([128, ncls], F32, tag="po")
                        nc.tensor.matmul(pc[:], finT[:], linw_s[:],
                                         start=True, stop=False,
                                         skip_group_check=True)
                        nc.tensor.matmul(pc[:], ones_row[:], linb_s[:],
                                         start=False, stop=True,
                                         skip_group_check=True)
                        ow = sbm.tile([128, ncls], F32, tag="ow")
                        nc.vector.tensor_copy(ow[:], pc[:])
                        nc.sync.dma_start(out_d[base:base + ws, :],
                                          ow[:ws, :])

                if li < L - 1:
                    table = dp.tile([n_nodes, hid], F32, tag="table",
                                    addr_space="Shared")
                    nc.gpsimd.collective_compute(
                        "AllGather", ALU.bypass, ins=[ag_next.opt()],
                        outs=[table.opt()], replica_groups=rg)
                    ag = ag_next

    nc.compile()
    return nc


# --------------------------------------------------------------------------
# entry point
# --------------------------------------------------------------------------

_CACHE = {}


def _get_program(T, W, npc):
    key = (T, W, npc)
    if key not in _CACHE:
        _CACHE[key] = build_program(T, W, npc)
    return _CACHE[key]


def _install_ntff_hook():
    """Bridge trn_agent_boot's ctypes NTFF profiler into antenv.axon_hooks
    (absent from this image) so run_bass_kernel_spmd(trace=True) works."""
    import types

    if "antenv.axon_hooks" in sys.modules:
        return
    try:
        sys.path.insert(0, "/root/.axon_site")
        from trn_agent_boot.trn_boot import _ntff_profile_via_ctypes

        hook = _ntff_profile_via_ctypes("/opt/axon/libaxon_pjrt.so")
    except Exception:
        hook = None
    m = types.ModuleType("antenv.axon_hooks")
    state = {"hook": hook}
    m.get_axon_ntff_profile_hook = lambda: state["hook"]
    m.set_axon_ntff_profile_hook = lambda h: state.update(hook=h)
    sys.modules["antenv.axon_hooks"] = m
    import antenv

    antenv.axon_hooks = m


def run(inputs, trace=False):
    if trace:
        _install_ntff_hook()
    T, W, npc, in_maps = _prep_inputs(inputs)
    nc = _get_program(T, W, npc)
    res = run_bass_kernel_spmd(nc, in_maps, list(range(CORES)), trace=trace)
    out = np.concatenate([res.results[c]["out"] for c in range(CORES)], axis=0)
    return out, res


def kernel(**inputs) -> np.ndarray:
    out, _ = run(inputs, trace=False)
    return out


# revision 23
# speedup vs baseline: 2.2628x; 1.0440x over previous
"""CGCNN / GENConv GNN message-passing kernel for 8 Trainium2 NeuronCores.

Strategy (dst-sharded edge parallel):
  - Host sorts edges by dst and shards them by dst node range: core k owns
    nodes [k*3750, (k+1)*3750) and every edge pointing into that range.
    Segment softmax/sums therefore never cross cores.
  - Each layer: every core gathers h[src] for its edges from a replicated
    node table in its local DRAM (dma_gather, 512B rows), computes
    msg = relu(h_src + ea) + eps, e = exp(t*msg), me = msg*e, and
    segment-reduces [sum e | sum me] per 128-node window with a one-hot
    matmul accumulated in PSUM.  alpha-normalization folds into a single
    per-node divide: agg = (sum me) / (sum e + eps)  (exactly equal to the
    reference softmax aggregation up to ~1e-16: the max-subtraction in the
    reference cancels algebraically and logits here are O(1)).
  - Node MLP / LayerNorm runs data-parallel on the core's own node shard.
  - The updated table (conv input of the next layer) is AllGather'ed
    across the 8 cores (~1.9MB per rank).

kernel(**inputs) takes the FULL inputs and returns the FULL [30000, 10]
output; sharding + compilation happen inside (compiled program is cached).
"""

import os
import sys

sys.path.insert(0, "/opt/trn_rl_repo")

ME_ENGINE = os.environ.get("K_ME_ENGINE", "vector")  # gpsimd | vector
NO_GATHER = os.environ.get("K_NO_GATHER", "0") == "1"
NO_ONEHOT = os.environ.get("K_NO_ONEHOT", "0") == "1"
SKIP_MLP = os.environ.get("K_SKIP_MLP", "0") == "1"
NO_INPLACE = os.environ.get("K_NO_INPLACE", "0") == "1"
MLP_STAGE = int(os.environ.get("K_MLP_STAGE", "9"))
LN_STAGE = int(os.environ.get("K_LN_STAGE", "9"))

import numpy as np

import concourse.bacc as bacc
import concourse.bass as bass
import concourse.mybir as mybir
import concourse.tile as tile
from concourse.bass_utils import run_bass_kernel_spmd
from concourse.library_config import mlp as mlp_lib

F32 = mybir.dt.float32
F16 = mybir.dt.float16
F32R = mybir.dt.float32r
I32 = mybir.dt.int32
AF = mybir.ActivationFunctionType
ALU = mybir.AluOpType

MSG_EPS = 1e-7
SM_EPS = 1e-16
LN_EPS = 1e-5

# problem dims (hardcoded per harness contract)
N_NODES = 30000
N_EDGES = 480000
F_IN = 64
F_EDGE = 16
HID = 128
N_LAYERS = 3
N_CLASSES = 10
CORES = 8


# --------------------------------------------------------------------------
# host-side sharding / packing
# --------------------------------------------------------------------------

def _prep_edges(edge_index, edge_attr, n_nodes, cores, npc, win):
    """Sort edges by dst, shard by dst range, pack per (core, window, tile).

    win = nodes per window (128).  Returns (T, per-core dict arrays).
    """
    src = edge_index[0].astype(np.int64)
    dst = edge_index[1].astype(np.int64)
    order = np.argsort(dst, kind="stable")
    src = src[order]
    dst = dst[order]
    attr = edge_attr[order]

    W = (npc + win - 1) // win  # windows per core
    # window id of each edge globally: core * W + (local node // win)
    core_of = dst // npc
    wloc = (dst - core_of * npc) // win
    gwin = core_of * W + wloc
    # edges are sorted by dst so gwin is non-decreasing
    counts = np.bincount(gwin, minlength=cores * W)
    T = int(np.ceil(counts.max() / 128.0))
    T = max(T, 1)
    epw = T * 128  # padded edges per window
    EPAD = W * epw

    starts = np.zeros(cores * W + 1, np.int64)
    np.cumsum(counts, out=starts[1:])

    fe = attr.shape[1]
    src_pad = np.zeros((cores, W, epw), np.int64)
    dstloc_pad = np.full((cores, W, epw), -1.0, np.float32)
    attr_pad = np.zeros((cores, W, epw, fe), np.float32)
    for c in range(cores):
        for w in range(W):
            g = c * W + w
            s, e = starts[g], starts[g + 1]
            n = e - s
            src_pad[c, w, :n] = src[s:e]
            dstloc_pad[c, w, :n] = (dst[s:e] - (c * npc + w * win)).astype(
                np.float32
            )
            attr_pad[c, w, :n, :] = attr[s:e]

    # gather index layout: [128, W*T*8] int16, idx i of window w at
    # partition i%16 (replicated x8), column w*T*8 + i//16
    sp = src_pad.reshape(cores, W, T * 8, 16)
    gidx16 = np.transpose(sp, (0, 3, 1, 2)).reshape(cores, 16, W * T * 8)
    gidx = np.tile(gidx16, (1, 8, 1)).astype(np.int16)

    # dstloc: [128, W*T] f32, tile j=w*T+g column, partition = edge offset
    dl = dstloc_pad.reshape(cores, W, T, 128)
    dstloc = np.ascontiguousarray(
        np.transpose(dl, (0, 3, 1, 2)).reshape(cores, 128, W * T)
    )

    # attrT: [17, EPAD]: rows 0..15 features (transposed), row 16 = ones
    ap = attr_pad.reshape(cores, EPAD, fe)
    attrT = np.empty((cores, fe + 1, EPAD), np.float16)
    attrT[:, :fe, :] = np.transpose(ap, (0, 2, 1))
    attrT[:, fe, :] = 1.0
    return T, W, gidx, dstloc, np.ascontiguousarray(attrT)


def _prep_inputs(inputs, cores=CORES):
    """Build the 8 per-core input maps (and shared weight arrays)."""
    x = np.asarray(inputs["x"], np.float32)
    edge_attr = np.asarray(inputs["edge_attr"], np.float32)
    edge_index = np.asarray(inputs["edge_index"])
    n_nodes, fin = x.shape
    npc = n_nodes // cores
    win = 128

    T, W, gidx, dstloc, attrT = _prep_edges(
        edge_index, edge_attr, n_nodes, cores, npc, win
    )

    L = int(np.asarray(inputs["t"]).shape[0])
    hid = np.asarray(inputs["node_enc_w"]).shape[1]

    # xT per core: [fin+1, npc] with ones row
    xs = x.reshape(cores, npc, fin)
    xT = np.empty((cores, fin + 1, npc), np.float32)
    xT[:, :fin, :] = np.transpose(xs, (0, 2, 1))
    xT[:, fin, :] = 1.0

    wnode = np.concatenate(
        [np.asarray(inputs["node_enc_w"], np.float32),
         np.asarray(inputs["node_enc_b"], np.float32)[None, :]], 0
    )
    wenc = np.concatenate(
        [np.asarray(inputs["edge_enc_w"], np.float32),
         np.asarray(inputs["edge_enc_b"], np.float32)[None, :]], 0
    )
    w1 = np.ascontiguousarray(inputs["mlp1_w"], np.float32)      # [L,H,2H]
    w2 = np.ascontiguousarray(inputs["mlp2_w"], np.float32)      # [L,2H,H]
    b1 = np.ascontiguousarray(
        np.asarray(inputs["mlp1_b"], np.float32).reshape(1, -1))  # [1,L*2H]
    b2 = np.ascontiguousarray(
        np.asarray(inputs["mlp2_b"], np.float32).reshape(1, -1))  # [1,L*H]
    g1bc = np.ascontiguousarray(
        np.broadcast_to(np.asarray(inputs["mlp_ln_g"], np.float16)[:, None, :],
                        (L, 128, 2 * hid)))
    bb1bc = np.ascontiguousarray(
        np.broadcast_to(np.asarray(inputs["mlp_ln_b"], np.float16)[:, None, :],
                        (L, 128, 2 * hid)))
    ngbc = np.ascontiguousarray(
        np.broadcast_to(np.asarray(inputs["norm_g"], np.float32)[:, None, :],
                        (L, 128, hid)))
    nbbc = np.ascontiguousarray(
        np.broadcast_to(np.asarray(inputs["norm_b"], np.float32)[:, None, :],
                        (L, 128, hid)))
    tcol = np.ascontiguousarray(
        np.broadcast_to(np.asarray(inputs["t"], np.float32)[None, :], (128, L)))
    linw = np.ascontiguousarray(inputs["lin_w"], np.float32)
    linb = np.ascontiguousarray(
        np.asarray(inputs["lin_b"], np.float32)[None, :])
    iota = np.ascontiguousarray(
        np.broadcast_to(np.arange(128, dtype=np.float32)[None, :], (128, 128)))
    ident = np.eye(128, dtype=np.float32)

    wench = wenc.astype(np.float16)
    w1h = w1.astype(np.float16)
    w2h = w2.astype(np.float16)
    linwh = linw.astype(np.float16)
    shared = dict(wnode=wnode, wench=wench, w1h=w1h, w2h=w2h, b1=b1, b2=b2,
                  g1bc=g1bc, bb1bc=bb1bc, ngbc=ngbc, nbbc=nbbc, tcol=tcol,
                  linwh=linwh, linb=linb, iota=iota, ident=ident)
    in_maps = []
    for c in range(cores):
        m = dict(shared)
        m["xT"] = np.ascontiguousarray(xT[c])
        m["attrT"] = attrT[c]
        m["gidx"] = np.ascontiguousarray(gidx[c])
        m["dstloc"] = dstloc[c]
        in_maps.append(m)
    return T, W, npc, in_maps


# --------------------------------------------------------------------------
# device program
# --------------------------------------------------------------------------

def build_program(T, W, npc, n_nodes=N_NODES, cores=CORES, fin=F_IN,
                  fe=F_EDGE, hid=HID, L=N_LAYERS, ncls=N_CLASSES):
    EPAD = W * T * 128
    H2 = 2 * hid
    nc = bacc.Bacc("TRN2", target_bir_lowering=False, debug=False,
                   num_devices=cores, dynamic_dma_scratch_size=131072,
                   num_swdge_queues=4)

    d = nc.dram_tensor
    xT_d = d("xT", [fin + 1, npc], F32, kind="ExternalInput")
    attrT_d = d("attrT", [fe + 1, EPAD], F16, kind="ExternalInput")
    gidx_d = d("gidx", [128, W * T * 8], mybir.dt.int16, kind="ExternalInput")
    dstloc_d = d("dstloc", [128, W * T], F32, kind="ExternalInput")
    wnode_d = d("wnode", [fin + 1, hid], F32, kind="ExternalInput")
    wenc_d = d("wench", [fe + 1, hid], F16, kind="ExternalInput")
    w1_d = d("w1h", [L, hid, H2], F16, kind="ExternalInput")
    w2_d = d("w2h", [L, H2, hid], F16, kind="ExternalInput")
    b1_d = d("b1", [1, L * H2], F32, kind="ExternalInput")
    b2_d = d("b2", [1, L * hid], F32, kind="ExternalInput")
    g1bc_d = d("g1bc", [L, 128, H2], F16, kind="ExternalInput")
    bb1bc_d = d("bb1bc", [L, 128, H2], F16, kind="ExternalInput")
    ngbc_d = d("ngbc", [L, 128, hid], F32, kind="ExternalInput")
    nbbc_d = d("nbbc", [L, 128, hid], F32, kind="ExternalInput")
    tcol_d = d("tcol", [128, L], F32, kind="ExternalInput")
    linw_d = d("linwh", [hid, ncls], F16, kind="ExternalInput")
    linb_d = d("linb", [1, ncls], F32, kind="ExternalInput")
    iota_d = d("iota", [128, 128], F32, kind="ExternalInput")
    ident_d = d("ident", [128, 128], F32, kind="ExternalInput")
    out_d = d("out", [npc, ncls], F32, kind="ExternalOutput")

    rg = [list(range(cores))]

    with tile.TileContext(nc) as tc:
        nc.gpsimd.load_library(mlp_lib)
        with (
            tc.tile_pool(name="const", bufs=1) as cp,
            tc.tile_pool(name="sbw", bufs=2) as sbw,       # window tiles
            tc.tile_pool(name="sbm", bufs=2) as sbm,       # MLP scratch
            tc.tile_pool(name="psq", bufs=2, space="PSUM") as psq,
            tc.tile_pool(name="psa", bufs=2, space="PSUM") as psa,
            tc.tile_pool(name="psu", bufs=2, space="PSUM") as psu,
            tc.tile_pool(name="pst", bufs=1, space="PSUM") as pst,
            tc.tile_pool(name="pso", bufs=1, space="PSUM") as pso,
            tc.tile_pool(name="dram", bufs=2, space="DRAM") as dp,
            tc.tile_pool(name="dram1", bufs=1, space="DRAM") as dp1,
        ):
            # ---------------- constants / weights to SBUF ----------------
            def load(name, dram_ap, shape, rearr=None, dt_=F32, **kw):
                t = cp.tile(shape, dt_, tag=name)
                src = dram_ap if rearr is None else dram_ap.rearrange(rearr, **kw)
                nc.sync.dma_start(t[:], src)
                return t

            wnode_s = load("wnode", wnode_d[:, :], [fin + 1, hid])
            wenc_s = load("wenc", wenc_d[:, :], [fe + 1, hid], dt_=F16)
            w1_s = load("w1", w1_d[:, :, :], [hid, L, H2], "l k n -> k l n",
                        dt_=F16)
            w2_s = load("w2", w2_d[:, :, :], [128, L, 2, hid],
                        "l (h k) n -> k l h n", h=2, dt_=F16)
            b1_s = load("b1", b1_d[:, :], [1, L * H2])
            b2_s = load("b2", b2_d[:, :], [1, L * hid])
            g1_s = load("g1", g1bc_d[:, :, :], [128, L, H2], "l p n -> p l n",
                        dt_=F16)
            bb1_s = load("bb1", bb1bc_d[:, :, :], [128, L, H2],
                         "l p n -> p l n", dt_=F16)
            ng_s = load("ng", ngbc_d[:, :, :], [128, L, hid], "l p n -> p l n")
            nb_s = load("nb", nbbc_d[:, :, :], [128, L, hid], "l p n -> p l n")
            tcol_s = load("tcol", tcol_d[:, :], [128, L])
            linw_s = load("linw", linw_d[:, :], [hid, ncls], dt_=F16)
            linb_s = load("linb", linb_d[:, :], [1, ncls])
            iota_s = load("iota", iota_d[:, :], [128, 128])
            ident_s = load("ident", ident_d[:, :], [128, 128])
            dstloc_s = load("dstloc", dstloc_d[:, :], [128, W * T])
            gidx_s = cp.tile([128, W * T * 8], mybir.dt.int16, tag="gidx")
            nc.sync.dma_start(gidx_s[:], gidx_d[:, :])

            eps_col = cp.tile([128, 1], F32, tag="epsc")
            nc.vector.memset(eps_col[:], MSG_EPS)
            sm_col = cp.tile([128, 1], F32, tag="smc")
            nc.vector.memset(sm_col[:], SM_EPS)
            ln_col = cp.tile([128, 1], F32, tag="lnc")
            nc.vector.memset(ln_col[:], LN_EPS)
            ones_row = cp.tile([1, 128], F32, tag="ones")
            nc.vector.memset(ones_row[:], 1.0)

            h_state = dp1.tile([npc, hid], F32)

            # ---------------- helpers ----------------
            def layer_norm_relu(dst, src_ap, nfeat, g_ap, b_ap, sq_tag):
                """dst <- relu(LN(src) * g + b).  src may be PSUM."""
                ssum = sbm.tile([128, 1], F32, tag="lnsum")
                nc.vector.reduce_sum(ssum[:], src_ap, axis=mybir.AxisListType.X)
                mcol = sbm.tile([128, 1], F32, tag="lnm")
                nc.scalar.mul(mcol[:], ssum[:], 1.0 / nfeat)
                xm = sbm.tile([128, nfeat], F32, tag="lnxm" + sq_tag)
                mb = mcol[:].rearrange("p (o f) -> p o f", o=1).broadcast_to(
                    [128, 1, nfeat])
                nc.vector.tensor_tensor(
                    xm[:].rearrange("p (o f) -> p o f", o=1), src_ap.rearrange(
                        "p (o f) -> p o f", o=1), mb, op=ALU.subtract)
                if LN_STAGE == 2:
                    nc.scalar.activation(dst, xm[:], AF.Relu)
                    return
                sq = sbm.tile([128, nfeat], F32, tag="lnsq" + sq_tag)
                vsum = sbm.tile([128, 1], F32, tag="lnv")
                nc.vector.tensor_mul(sq[:], xm[:], xm[:])
                nc.vector.reduce_sum(vsum[:], sq[:], axis=mybir.AxisListType.X)
                if LN_STAGE == 3:
                    nc.scalar.activation(dst, sq[:], AF.Relu)
                    return
                # rstd = rsqrt(v/nfeat + eps): Quake seed + 2 Newton steps
                a_t = sbm.tile([128, 1], F32, tag="lnva")
                nc.vector.tensor_scalar(a_t[:], vsum[:], 1.0 / nfeat, LN_EPS,
                                        op0=ALU.mult, op1=ALU.add)
                g_t = sbm.tile([128, 1], F32, tag="lnq1")
                nc.vector.tensor_scalar(g_t[:].bitcast(I32),
                                        a_t[:].bitcast(I32), 1, None,
                                        op0=ALU.arith_shift_right)
                g2_t = sbm.tile([128, 1], F32, tag="lnq2")
                nc.vector.tensor_scalar(g2_t[:].bitcast(I32),
                                        g_t[:].bitcast(I32), -1, 0x5f3759df,
                                        op0=ALU.mult, op1=ALU.add)
                rstd = g2_t
                for _ in range(2):
                    gg = sbm.tile([128, 1], F32, tag="lnq3")
                    nc.vector.tensor_mul(gg[:], rstd[:], rstd[:])
                    nc.vector.tensor_mul(gg[:], gg[:], a_t[:])
                    nc.vector.tensor_scalar(gg[:], gg[:], -0.5, 1.5,
                                            op0=ALU.mult, op1=ALU.add)
                    gn = sbm.tile([128, 1], F32, tag="lnq4")
                    nc.vector.tensor_mul(gn[:], rstd[:], gg[:])
                    rstd = gn
                if LN_STAGE == 4:
                    nc.vector.tensor_scalar_mul(xm[:], xm[:], rstd[:])
                    nc.scalar.activation(dst, xm[:], AF.Relu)
                    return
                y = sbm.tile([128, nfeat], F32, tag="lny" + sq_tag)
                rb = rstd[:].rearrange("p (o f) -> p o f", o=1).broadcast_to(
                    [128, 1, nfeat])
                nc.vector.tensor_tensor(
                    y[:].rearrange("p (o f) -> p o f", o=1),
                    xm[:].rearrange("p (o f) -> p o f", o=1), rb, op=ALU.mult)
                if NO_INPLACE:
                    y2 = sbm.tile([128, nfeat], F32, tag="lnyy" + sq_tag)
                    nc.vector.tensor_mul(y2[:], y[:], g_ap)
                    y3 = sbm.tile([128, nfeat], F32, tag="lnyz" + sq_tag)
                    nc.vector.tensor_add(y3[:], y2[:], b_ap)
                    nc.scalar.activation(dst, y3[:], AF.Relu)
                else:
                    nc.vector.tensor_mul(y[:], y[:], g_ap)
                    nc.vector.tensor_add(y[:], y[:], b_ap)
                    nc.scalar.activation(dst, y[:], AF.Relu)

            def transpose128(src_ap, tag, dt_=F16):
                """PE transpose [128,128] -> SBUF (cast on copy-out)."""
                pt = pst.tile([128, 128], F32, tag="pt")
                nc.tensor.transpose(pt[:], src_ap, ident_s[:])
                st = sbm.tile([128, 128], dt_, tag=tag)
                nc.vector.tensor_copy(st[:], pt[:])
                return st

            # ---------------- encode phase: h0 = x @ wnode ----------------
            def new_ag(nm):
                return dp.tile([npc, hid], F16, tag="ag", name=nm)

            def ag_rows(agp, base, ws):
                return agp[base:base + ws, :]

            def new_table(nm):
                return dp.tile([n_nodes, hid], F16, tag="table",
                               addr_space="Shared", name=nm)

            def emit_ag(agp, table):
                nc.gpsimd.collective_compute(
                    "AllGather", ALU.bypass, ins=[agp.opt()],
                    outs=[table.opt()], replica_groups=rg)

            ag = new_ag("age")
            table = new_table("tbl0")
            for w in range(W):
                base = w * 128
                ws = min(128, npc - base)
                xts = sbm.tile([fin + 1, 128], F32, tag="xts")
                nc.sync.dma_start(xts[:, :ws], xT_d[:, base:base + ws])
                ph = pso.tile([128, hid], F32, tag="po")
                nc.tensor.matmul(ph[:ws, :], xts[:, :ws], wnode_s[:],
                                 start=True, stop=True)
                h0 = sbm.tile([128, hid], F16, tag="h0")
                nc.vector.tensor_copy(h0[:ws, :], ph[:ws, :])
                nc.sync.dma_start(ag_rows(ag, base, ws), h0[:ws, :])
            emit_ag(ag, table)

            # ---------------- conv layers ----------------
            NQ = (T + 3) // 4  # quads of <=4 tiles

            for li in range(L):
                ag_next = new_ag(f"agn{li}") if li < L - 1 else None
                next_table = new_table(f"tbl{li + 1}") if li < L - 1 else None
                for w in range(W):
                    base = w * 128
                    ws = min(128, npc - base)
                    jw = w * T

                    # gather h[src] for this window: [128, T, 128]
                    # (split into <=1024-index chunks: the SWDGE gather
                    # ucode wedges the device above ~1024 descriptors)
                    hsrc = sbw.tile([128, T, 128], F16, tag="hsrc", bufs=2)
                    if NO_GATHER:
                        nc.vector.memset(hsrc[:], 0.01)
                    else:
                        for qi, c0 in enumerate(range(0, T, 8)):
                            ct = min(8, T - c0)
                            nc.gpsimd.dma_gather(
                                hsrc[:, c0:c0 + ct, :], table[:, :],
                                gidx_s[:, (w * T + c0) * 8:
                                       (w * T + c0 + ct) * 8],
                                ct * 128, ct * 128, hid,
                                queue_num=(w * 3 + qi) % 4)
                    attrs = sbw.tile([fe + 1, T, 128], F16, tag="attrs")
                    nc.sync.dma_start(
                        attrs[:], attrT_d[:, w * T * 128:(w + 1) * T * 128])

                    msg = sbw.tile([128, T, 128], F16, tag="msg")
                    em = sbw.tile([128, 2, T, 128], F16, tag="em", bufs=1)
                    # quads: ea matmul + gathered-h add (identity matmul)
                    for q in range(NQ):
                        q0 = q * 4
                        qs = min(4, T - q0)
                        pq = psq.tile([128, 4, 128], F32, tag="pq")
                        for j in range(qs):
                            nc.tensor.matmul(
                                pq[:, j, :], attrs[:, q0 + j, :], wenc_s[:],
                                start=(j == 0), stop=(j == qs - 1),
                                skip_group_check=True)
                        sc = sbw.tile([128, 4, 128], F32, tag="sc", bufs=2)
                        nc.vector.tensor_add(sc[:, :qs, :], pq[:, :qs, :],
                                             hsrc[:, q0:q0 + qs, :])
                        # msg = relu(ea + h_src + eps)
                        nc.scalar.activation(
                            msg[:, q0:q0 + qs, :], sc[:, :qs, :], AF.Relu,
                            bias=eps_col[:])
                    # e = exp(t * msg) ; me = msg * e
                    nc.scalar.activation(
                        em[:, 0, :, :], msg[:], AF.Exp,
                        scale=tcol_s[:, li:li + 1])
                    me_eng = nc.gpsimd if ME_ENGINE == "gpsimd" else nc.vector
                    me_eng.tensor_tensor(
                        em[:, 1, :, :], msg[:], em[:, 0, :, :], op=ALU.mult)
                    # one-hot S for the whole window
                    S = sbw.tile([128, T, 128], F16, tag="S")
                    if NO_ONEHOT:
                        nc.vector.memset(S[:], 0.0)
                    else:
                        iota_b = iota_s[:].rearrange(
                            "p (o f) -> p o f", o=1).broadcast_to([128, T, 128])
                        dl_b = dstloc_s[:, jw:jw + T].rearrange(
                            "p (t o) -> p t o", o=1).broadcast_to([128, T, 128])
                        nc.vector.tensor_tensor(S[:], iota_b, dl_b,
                                                op=ALU.is_equal)
                    # segment accumulate [sum e | sum me] -> [128, 256] psum
                    acc = psa.tile([128, 2, hid], F32, tag="acc")
                    for g in range(T):
                        nc.tensor.matmul(
                            acc[:, :, :], S[:, g, :], em[:, :, g, :],
                            start=(g == 0), stop=(g == T - 1))

                    # agg = (sum me) / (sum e + eps)
                    sep = sbm.tile([128, hid], F32, tag="sep")
                    nc.vector.tensor_scalar_add(sep[:], acc[:, 0, :], SM_EPS)
                    rcse = sbm.tile([128, hid], F32, tag="rcse")
                    nc.vector.reciprocal_approx_fast(rcse[:], sep[:])
                    z = sbm.tile([128, hid], F32, tag="z")
                    nc.vector.tensor_mul(z[:], acc[:, 1, :], rcse[:])
                    # z += conv input rows (this core's shard of table source)
                    zin = sbm.tile([128, hid], F16, tag="zin")
                    nc.sync.dma_start(zin[:ws, :], ag_rows(ag, base, ws))
                    if NO_INPLACE:
                        z2t = sbm.tile([128, hid], F32, tag="z2t")
                        nc.vector.tensor_add(z2t[:], z[:], zin[:])
                        z = z2t
                    else:
                        nc.vector.tensor_add(z[:], z[:], zin[:])

                    if SKIP_MLP:
                        nc.sync.dma_start(out_d[base:base + ws, :],
                                          z[:ws, :ncls])
                        continue

                    # ---- MLP: relu(LN(z@w1+b1))@w2+b2 ----
                    zT = transpose128(z[:], "zT")
                    if MLP_STAGE == 1:
                        nc.sync.dma_start(out_d[base:base + ws, :],
                                          zT[:ws, :ncls])
                        continue
                    pu = psu.tile([128, H2], F32, tag="pu")
                    nc.tensor.matmul(pu[:], zT[:], w1_s[:, li, :],
                                     start=True, stop=False,
                                     skip_group_check=True)
                    nc.tensor.matmul(pu[:], ones_row[:],
                                     b1_s[:, li * H2:(li + 1) * H2],
                                     start=False, stop=True,
                                     skip_group_check=True)
                    if MLP_STAGE == 2:
                        uu = sbm.tile([128, ncls], F32, tag="uu")
                        nc.vector.tensor_copy(uu[:], pu[:, :ncls])
                        nc.sync.dma_start(out_d[base:base + ws, :],
                                          uu[:ws, :])
                        continue
                    r = sbm.tile([128, H2], F32, tag="r")
                    layer_norm_relu(r[:], pu[:], H2, g1_s[:, li, :],
                                    bb1_s[:, li, :], "a")
                    if MLP_STAGE == 3:
                        nc.sync.dma_start(out_d[base:base + ws, :],
                                          r[:ws, :ncls])
                        continue
                    rT0 = transpose128(r[:, 0:128], "rT0")
                    rT1 = transpose128(r[:, 128:256], "rT1")
                    po = pso.tile([128, hid], F32, tag="po")
                    nc.tensor.matmul(po[:], rT0[:], w2_s[:, li, 0, :],
                                     start=True, stop=False,
                                     skip_group_check=True)
                    nc.tensor.matmul(po[:], rT1[:], w2_s[:, li, 1, :],
                                     start=False, stop=False,
                                     skip_group_check=True)
                    nc.tensor.matmul(po[:], ones_row[:],
                                     b2_s[:, li * hid:(li + 1) * hid],
                                     start=False, stop=True,
                                     skip_group_check=True)

                    if MLP_STAGE == 4:
                        oo = sbm.tile([128, ncls], F32, tag="oo")
                        nc.vector.tensor_copy(oo[:], po[:, :ncls])
                        nc.sync.dma_start(out_d[base:base + ws, :],
                                          oo[:ws, :])
                        continue
                    # ---- layer epilogue ----
                    hcur = sbm.tile([128, hid], F32, tag="hcur")
                    if li == 0:
                        nc.vector.tensor_copy(hcur[:], po[:])
                    else:
                        hprev = sbm.tile([128, hid], F32, tag="hprev")
                        nc.sync.dma_start(hprev[:ws, :],
                                          h_state[base:base + ws, :])
                        nc.vector.tensor_add(hcur[:], po[:], hprev[:])
                    if li < L - 1:
                        nc.sync.dma_start(h_state[base:base + ws, :],
                                          hcur[:ws, :])
                        # z for next layer: relu(LN(h; norm[li+1]))
                        znext = sbm.tile([128, hid], F16, tag="znext")
                        layer_norm_relu(znext[:], hcur[:], hid,
                                        ng_s[:, li + 1, :], nb_s[:, li + 1, :],
                                        "b")
                        nc.sync.dma_start(ag_rows(ag_next, base, ws),
                                          znext[:ws, :])

                    else:
                        # final: relu(LN(h; norm[0])) @ lin_w + lin_b
                        fin_t = sbm.tile([128, hid], F32, tag="fin")
                        layer_norm_relu(fin_t[:], hcur[:], hid,
                                        ng_s[:, 0, :], nb_s[:, 0, :], "b")
                        finT = transpose128(fin_t[:], "finT")
                        pc = pso.tile([128, ncls], F32, tag="po")
                        nc.tensor.matmul(pc[:], finT[:], linw_s[:],
                                         start=True, stop=False,
                                         skip_group_check=True)
                        nc.tensor.matmul(pc[:], ones_row[:], linb_s[:],
                                         start=False, stop=True,
                                         skip_group_check=True)
                        ow = sbm.tile([128, ncls], F32, tag="ow")
                        nc.vector.tensor_copy(ow[:], pc[:])
                        nc.sync.dma_start(out_d[base:base + ws, :],
                                          ow[:ws, :])

                if li < L - 1:
                    emit_ag(ag_next, next_table)
                    table = next_table
                    ag = ag_next

    nc.compile()
    return nc


# --------------------------------------------------------------------------
# entry point
# --------------------------------------------------------------------------

_CACHE = {}


def _get_program(T, W, npc):
    key = (T, W, npc)
    if key not in _CACHE:
        _CACHE[key] = build_program(T, W, npc)
    return _CACHE[key]


def _install_ntff_hook():
    """Bridge trn_agent_boot's ctypes NTFF profiler into antenv.axon_hooks
    (absent from this image) so run_bass_kernel_spmd(trace=True) works."""
    import types

    if "antenv.axon_hooks" in sys.modules:
        return
    try:
        sys.path.insert(0, "/root/.axon_site")
        from trn_agent_boot.trn_boot import _ntff_profile_via_ctypes

        hook = _ntff_profile_via_ctypes("/opt/axon/libaxon_pjrt.so")
    except Exception:
        hook = None
    m = types.ModuleType("antenv.axon_hooks")
    state = {"hook": hook}
    m.get_axon_ntff_profile_hook = lambda: state["hook"]
    m.set_axon_ntff_profile_hook = lambda h: state.update(hook=h)
    sys.modules["antenv.axon_hooks"] = m
    import antenv

    antenv.axon_hooks = m


def run(inputs, trace=False):
    if trace:
        _install_ntff_hook()
    T, W, npc, in_maps = _prep_inputs(inputs)
    nc = _get_program(T, W, npc)
    res = run_bass_kernel_spmd(nc, in_maps, list(range(CORES)), trace=trace)
    out = np.concatenate([res.results[c]["out"] for c in range(CORES)], axis=0)
    return out, res


def kernel(**inputs) -> np.ndarray:
    out, _ = run(inputs, trace=False)
    return out
